# revision 1
# baseline (speedup 1.0000x reference)
"""SSD detection post-processing (decode + softmax + per-class top-200 + NMS,
TTA-flip merge) as a Bass/Tile kernel for 8 Trainium2 NeuronCores.

Sharding: pure data parallel over the batch dim — core k handles images
8k..8k+7 (= 168 (image,class) pairs per core).

Per-core pipeline (all on device):
  1. decode both views' boxes (flip view 2), store to a DRAM box table
  2. softmax probs in prior-major layout; 32x32 stream-transpose + SBUF-SBUF
     DMA reassembly into pair-major score rows [pair, 17664]
  3. chunk-max (L=8) -> M [pair, 2208]; 25 rounds of max8/max_index/
     match_replace extract the 200 largest chunk maxes (provably a superset
     of the chunks holding the global top-200: if >200 chunks had max above
     the 201st value, there would be >200 elements above it)
  4. indirect-DMA gather of those chunks -> pool [pair, 1600]; 25 more
     extraction rounds give the exact sorted top-200 + pool slots
  5. map pool slots -> chunk ids -> score-column index; indirect-gather boxes
  6. pairwise IoU mask (iou > 0.45 as 1.45*inter - 0.45*(ai+aj) > 0)
  7. greedy sequential NMS scan (200 steps)
  8. compacted survivor rows scattered into the output via indirect DMA
"""

import numpy as np

B = 64
N = 8732
C = 21
NPAD = 8832            # priors padded to 69*128
SLOTS = NPAD // 128    # 69
W2 = 2 * NPAD          # 17664 score columns per pair
IMG = 8                # images per core
PAIRS = IMG * C        # 168 pairs per core
CHUNK = 8
NCHUNK = W2 // CHUNK   # 2208
ROUNDS = 25            # 25*8 = 200
K = 200
TILES = ((0, 126), (126, 42))   # (pair offset, pair count) per partition tile


def build_nc():
    import concourse.bacc as bacc
    import concourse.bass as bass
    import concourse.mybir as mybir
    from concourse.bass import IndirectOffsetOnAxis
    from concourse.tile import TileContext

    f32 = mybir.dt.float32
    u32 = mybir.dt.uint32
    u8 = mybir.dt.uint8
    Alu = mybir.AluOpType
    Act = mybir.ActivationFunctionType
    Ax = mybir.AxisListType

    nc = bacc.Bacc()

    def TT(out, in0, in1, op):
        # TensorTensor's ISA struct can't encode multiple sync waits (codegen
        # "Too many sync wait commands"); TensorScalarPtr can, so emit every
        # tensor-tensor op as (in0 bypass 0.0) op in1.
        nc.vector.scalar_tensor_tensor(
            out=out, in0=in0, scalar=0.0, in1=in1, op0=Alu.bypass, op1=op,
        )

    loc1 = nc.declare_dram_parameter("loc1", [IMG, NPAD, 4], f32, isOutput=False)
    loc2 = nc.declare_dram_parameter("loc2", [IMG, NPAD, 4], f32, isOutput=False)
    conf1 = nc.declare_dram_parameter("conf1", [IMG, NPAD, C], f32, isOutput=False)
    conf2 = nc.declare_dram_parameter("conf2", [IMG, NPAD, C], f32, isOutput=False)
    dbox = nc.declare_dram_parameter("dbox", [NPAD, 4], f32, isOutput=False)
    # aux[pair] = (img*W2 base row into box table, 1 if class==0 else 0)
    aux = nc.declare_dram_parameter("aux", [PAIRS, 2], u32, isOutput=False)
    outp = nc.declare_dram_parameter("out", [PAIRS * 201, 5], f32, isOutput=True)

    scoresD = nc.dram_tensor("scoresD", [PAIRS * NCHUNK, CHUNK], f32)
    boxesD = nc.dram_tensor("boxesD", [IMG * W2, 4], f32)
    cidsD = nc.dram_tensor("cidsD", [PAIRS * K, 1], u32)

    with TileContext(nc) as tc:
        with tc.tile_pool(name="Mpool", bufs=1) as mp:
            M_tiles = [
                mp.tile([cnt, NCHUNK], f32, tag=f"M{ti}", name=f"M{ti}")
                for ti, (off, cnt) in enumerate(TILES)
            ]
            # ---------------- phase 1: scores + boxes -----------------------
            with (
                tc.tile_pool(name="persist", bufs=1) as pp,
                tc.tile_pool(name="work", bufs=1) as wp,
            ):
                SA = pp.tile([126, W2], f32, tag="SA")
                SB = pp.tile([42, W2], f32, tag="SB")
                S_tiles = (SA, SB)

                dbox_t = pp.tile([128, SLOTS, 4], f32, tag="dbox")
                nc.sync.dma_start(
                    out=dbox_t[:, :, :],
                    in_=dbox.rearrange("(p s) c -> p s c", s=SLOTS),
                )

                for img in range(IMG):
                    for v, (locp, confp) in enumerate(
                        ((loc1, conf1), (loc2, conf2))
                    ):
                        # ---- decode ----
                        loc_t = wp.tile([128, SLOTS, 4], f32, tag="loc")
                        nc.sync.dma_start(
                            out=loc_t[:, :, :],
                            in_=locp[img].rearrange("(p s) c -> p s c", s=SLOTS),
                        )
                        box_t = wp.tile([128, SLOTS, 4], f32, tag="box")
                        wh_t = wp.tile([128, SLOTS, 2], f32, tag="wh")
                        t1_t = wp.tile([128, SLOTS, 2], f32, tag="dtmp")
                        # wh = dbox_wh * exp(0.2*loc_wh)
                        nc.scalar.activation(
                            wh_t[:, :, :], loc_t[:, :, 2:4], Act.Exp, scale=0.2
                        )
                        TT(
                            out=wh_t[:, :, :], in0=wh_t[:, :, :],
                            in1=dbox_t[:, :, 2:4], op=Alu.mult,
                        )
                        # cxy = dbox_xy + (loc_xy*0.1)*dbox_xy
                        nc.vector.tensor_scalar_mul(
                            t1_t[:, :, :], loc_t[:, :, :2], 0.1
                        )
                        TT(
                            out=t1_t[:, :, :], in0=t1_t[:, :, :],
                            in1=dbox_t[:, :, :2], op=Alu.mult,
                        )
                        TT(
                            out=t1_t[:, :, :], in0=t1_t[:, :, :],
                            in1=dbox_t[:, :, :2], op=Alu.add,
                        )
                        # mn = cxy - 0.5*wh ; mx = mn + wh
                        nc.vector.tensor_scalar_mul(
                            box_t[:, :, 2:4], wh_t[:, :, :], 0.5
                        )
                        TT(
                            out=box_t[:, :, 0:2], in0=t1_t[:, :, :],
                            in1=box_t[:, :, 2:4], op=Alu.subtract,
                        )
                        TT(
                            out=box_t[:, :, 2:4], in0=box_t[:, :, 0:2],
                            in1=wh_t[:, :, :], op=Alu.add,
                        )
                        if v == 1:
                            # flip: x1' = 1-x2, x2' = 1-x1
                            fx_t = wp.tile([128, SLOTS, 2], f32, tag="fx")
                            nc.vector.tensor_scalar(
                                fx_t[:, :, 0:1], box_t[:, :, 2:3], -1.0, 1.0,
                                op0=Alu.mult, op1=Alu.add,
                            )
                            nc.vector.tensor_scalar(
                                fx_t[:, :, 1:2], box_t[:, :, 0:1], -1.0, 1.0,
                                op0=Alu.mult, op1=Alu.add,
                            )
                            nc.vector.tensor_copy(
                                box_t[:, :, 0:1], fx_t[:, :, 0:1]
                            )
                            nc.vector.tensor_copy(
                                box_t[:, :, 2:3], fx_t[:, :, 1:2]
                            )
                        # boxesD row sigma = v*NPAD + pg*(SLOTS*32) + s*32 + l
                        for pg in range(4):
                            base = img * W2 + v * NPAD + pg * (SLOTS * 32)
                            nc.sync.dma_start(
                                out=boxesD[base:base + SLOTS * 32, :].rearrange(
                                    "(s l) c -> l s c", l=32
                                ),
                                in_=box_t[pg * 32:(pg + 1) * 32, :, :],
                            )

                        # ---- softmax (prior-major) ----
                        cf_t = wp.tile([128, SLOTS, C], f32, tag="cf")
                        nc.sync.dma_start(
                            out=cf_t[:, :, :],
                            in_=confp[img].rearrange("(p s) c -> p s c", s=SLOTS),
                        )
                        mx_t = wp.tile([128, SLOTS], f32, tag="mx")
                        nc.vector.tensor_reduce(
                            out=mx_t[:, :], in_=cf_t[:, :, :], axis=Ax.X,
                            op=Alu.max,
                        )
                        TT(
                            out=cf_t[:, :, :], in0=cf_t[:, :, :],
                            in1=mx_t[:, :, None].to_broadcast([128, SLOTS, C]),
                            op=Alu.subtract,
                        )
                        pr_t = wp.tile([128, SLOTS, 32], f32, tag="pr")
                        nc.vector.memset(pr_t[:, :, C:], 0.0)
                        nc.scalar.activation(
                            pr_t[:, :, :C], cf_t[:, :, :], Act.Exp
                        )
                        sm_t = wp.tile([128, SLOTS], f32, tag="sm")
                        nc.vector.tensor_reduce(
                            out=sm_t[:, :], in_=pr_t[:, :, :C], axis=Ax.X,
                            op=Alu.add,
                        )
                        nc.vector.reciprocal(sm_t[:, :], sm_t[:, :])
                        TT(
                            out=pr_t[:, :, :C], in0=pr_t[:, :, :C],
                            in1=sm_t[:, :, None].to_broadcast([128, SLOTS, C]),
                            op=Alu.mult,
                        )
                        # ---- 32x32 block transpose ----
                        tr_t = wp.tile([128, SLOTS, 32], f32, tag="tr")
                        nc.vector.transpose(
                            out=tr_t[:, :, :].rearrange("p s c -> p (s c)"),
                            in_=pr_t[:, :, :].rearrange("p s c -> p (s c)"),
                        )
                        # ---- SBUF->SBUF DMA into pair-major rows ----
                        if img < 6:
                            dst, row0 = SA, img * C
                        else:
                            dst, row0 = SB, (img - 6) * C
                        for pg in range(4):
                            nc.sync.dma_start(
                                out=dst[row0:row0 + C,
                                        v * NPAD + pg * (SLOTS * 32):
                                        v * NPAD + (pg + 1) * (SLOTS * 32)],
                                in_=tr_t[pg * 32: pg * 32 + C, :, :].rearrange(
                                    "c s l -> c (s l)"
                                ),
                            )

                # big copy of pair-major scores to DRAM + chunk max
                for (off, cnt), st, M_t in zip(TILES, S_tiles, M_tiles):
                    nc.sync.dma_start(
                        out=scoresD[off * NCHUNK:(off + cnt) * NCHUNK, :],
                        in_=st[:, :].rearrange("p (n k) -> p n k", k=CHUNK),
                    )
                    nc.vector.tensor_reduce(
                        out=M_t[:, :],
                        in_=st[:, :].rearrange("p (n k) -> p n k", k=CHUNK),
                        axis=Ax.X, op=Alu.max,
                    )
            # persist pool (SA/SB) freed here

            # zero the output (scatter only writes kept rows)
            with tc.tile_pool(name="zero", bufs=1) as zp:
                z_t = zp.tile([128, 201 * 5], f32, tag="z")
                nc.vector.memset(z_t[:, :], 0.0)
                for off, cnt in TILES:
                    nc.sync.dma_start(
                        out=outp[off * 201:(off + cnt) * 201, :].rearrange(
                            "(p r) c -> p (r c)", r=201),
                        in_=z_t[:cnt, :],
                    )

            # ---------------- phase 2: per-tile selection + NMS -------------
            with tc.tile_pool(name="sel", bufs=1) as sp:
              for ti, ((off, cnt), M_t) in enumerate(zip(TILES, M_tiles)):
                if True:
                    cids_t = sp.tile([cnt, K], u32, tag=f"cid{ti}")
                    v8 = sp.tile([cnt, 8], f32, tag=f"v8{ti}")
                    for r in range(ROUNDS):
                        nc.vector.max(out=v8[:, :], in_=M_t[:, :])
                        nc.vector.max_index(
                            out=cids_t[:, 8 * r:8 * r + 8],
                            in_max=v8[:, :], in_values=M_t[:, :],
                        )
                        nc.vector.match_replace(
                            out=M_t[:, :], in_to_replace=v8[:, :],
                            in_values=M_t[:, :], imm_value=-1.0,
                        )
                    nc.sync.dma_start(
                        out=cidsD[off * K:(off + cnt) * K, :],
                        in_=cids_t[:, :, None],
                    )
                    # gather the 200 selected chunks into the pool
                    base_t = sp.tile([cnt, 1], u32, tag=f"ba{ti}")
                    nc.gpsimd.iota(
                        base_t[:, :], pattern=[[0, 1]],
                        base=off * NCHUNK, channel_multiplier=NCHUNK,
                    )
                    gidx_t = sp.tile([cnt, K], u32, tag=f"gi{ti}")
                    TT(
                        out=gidx_t[:, :], in0=cids_t[:, :],
                        in1=base_t[:, :].to_broadcast([cnt, K]), op=Alu.add,
                    )
                    pool_t = sp.tile([cnt, K, CHUNK], f32, tag=f"po{ti}")
                    for sg in range(K):
                        nc.gpsimd.indirect_dma_start(
                            out=pool_t[:, sg, :], out_offset=None,
                            in_=scoresD[:, :],
                            in_offset=IndirectOffsetOnAxis(
                                ap=gidx_t[:, sg:sg + 1], axis=0),
                        )
                    # exact sorted top-200 from the pool
                    sorted_t = sp.tile([cnt, K], f32, tag=f"so{ti}")
                    ps_t = sp.tile([cnt, K], u32, tag=f"ps{ti}")
                    poolf = pool_t[:, :, :].rearrange("p n k -> p (n k)")
                    for r in range(ROUNDS):
                        nc.vector.max(
                            out=sorted_t[:, 8 * r:8 * r + 8], in_=poolf
                        )
                        nc.vector.max_index(
                            out=ps_t[:, 8 * r:8 * r + 8],
                            in_max=sorted_t[:, 8 * r:8 * r + 8],
                            in_values=poolf,
                        )
                        nc.vector.match_replace(
                            out=poolf,
                            in_to_replace=sorted_t[:, 8 * r:8 * r + 8],
                            in_values=poolf, imm_value=-1.0,
                        )
                    # pool slot -> chunk id (DRAM bounce) -> sigma column
                    sh_t = sp.tile([cnt, K], u32, tag=f"sh{ti}")
                    nc.vector.tensor_scalar(
                        sh_t[:, :], ps_t[:, :], 3, None,
                        op0=Alu.logical_shift_right,
                    )
                    b2_t = sp.tile([cnt, 1], u32, tag=f"b2{ti}")
                    nc.gpsimd.iota(
                        b2_t[:, :], pattern=[[0, 1]],
                        base=off * K, channel_multiplier=K,
                    )
                    g2_t = sp.tile([cnt, K], u32, tag=f"g2{ti}")
                    TT(
                        out=g2_t[:, :], in0=sh_t[:, :],
                        in1=b2_t[:, :].to_broadcast([cnt, K]), op=Alu.add,
                    )
                    csel_t = sp.tile([cnt, K, 1], u32, tag=f"cs{ti}")
                    for sg in range(K):
                        nc.gpsimd.indirect_dma_start(
                            out=csel_t[:, sg, :], out_offset=None,
                            in_=cidsD[:, :],
                            in_offset=IndirectOffsetOnAxis(
                                ap=g2_t[:, sg:sg + 1], axis=0),
                        )
                    # sigma = cid*8 + (ps - (ps>>3)<<3)
                    lane_t = sp.tile([cnt, K], u32, tag=f"la{ti}")
                    nc.vector.tensor_scalar(
                        lane_t[:, :], sh_t[:, :], 3, None,
                        op0=Alu.logical_shift_left,
                    )
                    TT(
                        out=lane_t[:, :], in0=ps_t[:, :], in1=lane_t[:, :],
                        op=Alu.subtract,
                    )
                    sig_t = sp.tile([cnt, K], u32, tag=f"sg{ti}")
                    nc.vector.tensor_scalar(
                        sig_t[:, :], csel_t[:, :, 0], 3, None,
                        op0=Alu.logical_shift_left,
                    )
                    TT(
                        out=sig_t[:, :], in0=sig_t[:, :], in1=lane_t[:, :],
                        op=Alu.add,
                    )
                    # box row = sigma + img*W2 (aux col 0)
                    aux_t = sp.tile([cnt, 2], u32, tag=f"ax{ti}")
                    nc.sync.dma_start(
                        out=aux_t[:, :], in_=aux[off:off + cnt, :]
                    )
                    TT(
                        out=sig_t[:, :], in0=sig_t[:, :],
                        in1=aux_t[:, 0:1].to_broadcast([cnt, K]), op=Alu.add,
                    )
                    bx_t = sp.tile([cnt, K, 4], f32, tag=f"bx{ti}")
                    for sg in range(K):
                        nc.gpsimd.indirect_dma_start(
                            out=bx_t[:, sg, :], out_offset=None,
                            in_=boxesD[:, :],
                            in_offset=IndirectOffsetOnAxis(
                                ap=sig_t[:, sg:sg + 1], axis=0),
                        )

                    # ---- IoU mask: S[i,j] = 1.45*inter - 0.45*(ai+aj) > 0 ----
                    ar_t = sp.tile([cnt, K], f32, tag=f"ar{ti}")
                    w0_t = sp.tile([cnt, K], f32, tag=f"w0{ti}")
                    TT(
                        out=w0_t[:, :], in0=bx_t[:, :, 2], in1=bx_t[:, :, 0],
                        op=Alu.subtract,
                    )
                    TT(
                        out=ar_t[:, :], in0=bx_t[:, :, 3], in1=bx_t[:, :, 1],
                        op=Alu.subtract,
                    )
                    TT(
                        out=ar_t[:, :], in0=ar_t[:, :], in1=w0_t[:, :],
                        op=Alu.mult,
                    )
                    Sm_t = sp.tile([cnt, K, K], u8, tag=f"Sm{ti}")
                    xa = sp.tile([cnt, 8, K], f32, tag="xa", name="xa")
                    xb = sp.tile([cnt, 8, K], f32, tag="xb", name="xb")
                    xc = sp.tile([cnt, 8, K], f32, tag="xc", name="xc")
                    for bi in range(K // 8):
                        r0 = 8 * bi
                        rows = bx_t[:, r0:r0 + 8, :]
                        TT(
                            out=xa[:, :, :],
                            in0=rows[:, :, 0:1].to_broadcast([cnt, 8, K]),
                            in1=bx_t[:, None, :, 0].to_broadcast([cnt, 8, K]),
                            op=Alu.max,
                        )
                        TT(
                            out=xb[:, :, :],
                            in0=rows[:, :, 2:3].to_broadcast([cnt, 8, K]),
                            in1=bx_t[:, None, :, 2].to_broadcast([cnt, 8, K]),
                            op=Alu.min,
                        )
                        TT(
                            out=xa[:, :, :], in0=xb[:, :, :], in1=xa[:, :, :],
                            op=Alu.subtract,
                        )
                        nc.vector.tensor_scalar_max(
                            xa[:, :, :], xa[:, :, :], 0.0
                        )
                        TT(
                            out=xc[:, :, :],
                            in0=rows[:, :, 1:2].to_broadcast([cnt, 8, K]),
                            in1=bx_t[:, None, :, 1].to_broadcast([cnt, 8, K]),
                            op=Alu.max,
                        )
                        TT(
                            out=xb[:, :, :],
                            in0=rows[:, :, 3:4].to_broadcast([cnt, 8, K]),
                            in1=bx_t[:, None, :, 3].to_broadcast([cnt, 8, K]),
                            op=Alu.min,
                        )
                        TT(
                            out=xb[:, :, :], in0=xb[:, :, :], in1=xc[:, :, :],
                            op=Alu.subtract,
                        )
                        nc.vector.tensor_scalar_max(
                            xb[:, :, :], xb[:, :, :], 0.0
                        )
                        TT(
                            out=xa[:, :, :], in0=xa[:, :, :], in1=xb[:, :, :],
                            op=Alu.mult,
                        )
                        TT(
                            out=xb[:, :, :],
                            in0=ar_t[:, r0:r0 + 8, None].to_broadcast(
                                [cnt, 8, K]),
                            in1=ar_t[:, None, :].to_broadcast([cnt, 8, K]),
                            op=Alu.add,
                        )
                        nc.vector.tensor_scalar_mul(
                            xa[:, :, :], xa[:, :, :], 1.45
                        )
                        nc.vector.scalar_tensor_tensor(
                            out=xa[:, :, :], in0=xb[:, :, :], scalar=-0.45,
                            in1=xa[:, :, :], op0=Alu.mult, op1=Alu.add,
                        )
                        nc.vector.tensor_scalar(
                            Sm_t[:, r0:r0 + 8, :], xa[:, :, :], 0.0, None,
                            op0=Alu.is_gt,
                        )

                    # ---- greedy NMS scan ----
                    keep_t = sp.tile([cnt, K], u8, tag=f"ke{ti}")
                    nc.vector.memset(keep_t[:, :], 0)
                    tmp_t = sp.tile([cnt, K], u8, tag=f"tm{ti}")
                    sup_t = sp.tile([cnt, 1], u8, tag=f"su{ti}")
                    for i in range(K):
                        TT(
                            out=tmp_t[:, :], in0=Sm_t[:, i, :],
                            in1=keep_t[:, :], op=Alu.mult,
                        )
                        nc.vector.tensor_reduce(
                            out=sup_t[:, :], in_=tmp_t[:, :], axis=Ax.X,
                            op=Alu.max,
                        )
                        nc.vector.tensor_scalar(
                            keep_t[:, i:i + 1], sup_t[:, :], 0, None,
                            op0=Alu.is_equal,
                        )

                    # ---- output scatter ----
                    keepf_t = sp.tile([cnt, K], f32, tag=f"kf{ti}")
                    nc.vector.tensor_copy(keepf_t[:, :], keep_t[:, :])
                    pos_t = sp.tile([cnt, K], f32, tag=f"pf{ti}")
                    nc.vector.tensor_tensor_scan(
                        out=pos_t[:, :], data0=keepf_t[:, :],
                        data1=keepf_t[:, :], initial=-1.0,
                        op0=Alu.add, op1=Alu.bypass,
                    )
                    posx_t = sp.tile([cnt, K], f32, tag=f"px{ti}")
                    nc.vector.memset(posx_t[:, :], float(K))
                    nc.vector.copy_predicated(
                        posx_t[:, :], keep_t[:, :], pos_t[:, :]
                    )
                    posu_t = sp.tile([cnt, K], u32, tag=f"pu{ti}")
                    nc.vector.tensor_copy(posu_t[:, :], posx_t[:, :])
                    # class-0 pairs always go to the trash row
                    cls0_t = sp.tile([cnt, K], u32, tag=f"c0{ti}")
                    nc.vector.tensor_scalar(
                        cls0_t[:, :],
                        aux_t[:, 1:2].to_broadcast([cnt, K]), K, None,
                        op0=Alu.mult,
                    )
                    TT(
                        out=posu_t[:, :], in0=posu_t[:, :], in1=cls0_t[:, :],
                        op=Alu.max,
                    )
                    b3_t = sp.tile([cnt, 1], u32, tag=f"b3{ti}")
                    nc.gpsimd.iota(
                        b3_t[:, :], pattern=[[0, 1]],
                        base=off * 201, channel_multiplier=201,
                    )
                    TT(
                        out=posu_t[:, :], in0=posu_t[:, :],
                        in1=b3_t[:, :].to_broadcast([cnt, K]), op=Alu.add,
                    )
                    row_t = sp.tile([cnt, K, 5], f32, tag=f"ro{ti}")
                    nc.vector.tensor_copy(row_t[:, :, 0], sorted_t[:, :])
                    nc.vector.tensor_copy(row_t[:, :, 1:5], bx_t[:, :, :])
                    for sg in range(K):
                        nc.gpsimd.indirect_dma_start(
                            out=outp[:, :],
                            out_offset=IndirectOffsetOnAxis(
                                ap=posu_t[:, sg:sg + 1], axis=0),
                            in_=row_t[:, sg, :], in_offset=None,
                        )
    nc.compile()
    return nc


def _prep_core_inputs(loc_b, conf_b, loc2_b, conf2_b, dbox):
    """Pad per-core inputs to NPAD priors; build aux table."""
    pad = NPAD - N
    locp = np.pad(loc_b, ((0, 0), (0, pad), (0, 0)))
    loc2p = np.pad(loc2_b, ((0, 0), (0, pad), (0, 0)))
    cpad = np.zeros((conf_b.shape[0], pad, C), np.float32)
    cpad[:, :, 0] = 40.0
    cpad[:, :, 1:] = -40.0
    confp = np.concatenate([conf_b, cpad], axis=1)
    conf2p = np.concatenate([conf2_b, cpad], axis=1)
    dpad = np.zeros((pad, 4), np.float32)
    dpad[:, 2:] = 1e-3
    dboxp = np.concatenate([dbox, dpad], axis=0)
    aux = np.zeros((PAIRS, 2), np.uint32)
    for p in range(PAIRS):
        aux[p, 0] = (p // C) * W2
        aux[p, 1] = 1 if (p % C) == 0 else 0
    return {
        "loc1": np.ascontiguousarray(locp, np.float32),
        "loc2": np.ascontiguousarray(loc2p, np.float32),
        "conf1": np.ascontiguousarray(confp, np.float32),
        "conf2": np.ascontiguousarray(conf2p, np.float32),
        "dbox": np.ascontiguousarray(dboxp, np.float32),
        "aux": aux,
    }


def kernel(loc_data, conf_data, loc_data2, conf_data2, dbox_list):
    from concourse.bass_utils import run_bass_kernel_spmd

    loc_data = np.asarray(loc_data, np.float32)
    conf_data = np.asarray(conf_data, np.float32)
    loc_data2 = np.asarray(loc_data2, np.float32)
    conf_data2 = np.asarray(conf_data2, np.float32)
    dbox_list = np.asarray(dbox_list, np.float32)

    nc = build_nc()
    in_maps = []
    for k in range(8):
        sl = slice(k * IMG, (k + 1) * IMG)
        in_maps.append(
            _prep_core_inputs(
                loc_data[sl], conf_data[sl], loc_data2[sl], conf_data2[sl],
                dbox_list,
            )
        )
    res = run_bass_kernel_spmd(nc, in_maps, list(range(8))).results
    outs = []
    for k in range(8):
        o = np.asarray(res[k]["out"]).reshape(PAIRS, 201, 5)[:, :K, :]
        outs.append(o.reshape(IMG, C, K, 5))
    return np.concatenate(outs, axis=0)



# revision 11
# speedup vs baseline: 8.0933x; 8.0933x over previous
"""SSD detection post-processing (decode + softmax + per-class top-200 + NMS,
TTA-flip merge) as a Bass/Tile kernel for 8 Trainium2 NeuronCores.

Sharding: pure data parallel over the batch dim — core k handles images
8k..8k+7 (= 168 (image,class) pairs per core).

Per-core pipeline (all on device):
  1. decode both views' boxes (flip view 2), store to a DRAM box table
  2. softmax probs in prior-major layout; 32x32 stream-transpose + SBUF-SBUF
     DMA reassembly into pair-major score rows [pair, 17664]
  3. chunk-max (L=8) -> M [pair, 2208]; 25 rounds of max8/max_index/
     match_replace extract the 200 largest chunk maxes (provably a superset
     of the chunks holding the global top-200: if >200 chunks had max above
     the 201st value, there would be >200 elements above it)
  4. indirect-DMA gather of those chunks -> pool [pair, 1600]; 25 more
     extraction rounds give the exact sorted top-200 + pool slots
  5. map pool slots -> chunk ids -> score-column index; indirect-gather boxes
  6. pairwise IoU mask (iou > 0.45 as 1.45*inter - 0.45*(ai+aj) > 0)
  7. greedy sequential NMS scan (200 steps)
  8. compacted survivor rows scattered into the output via indirect DMA
"""

import numpy as np

B = 64
N = 8732
C = 21
NPAD = 8832            # priors padded to 69*128
SLOTS = NPAD // 128    # 69
W2 = 2 * NPAD          # 17664 score columns per pair
IMG = 8                # images per core
PAIRS = IMG * C        # 168 pairs per core
CHUNK = 8
NCHUNK = W2 // CHUNK   # 2208
ROUNDS = 25            # 25*8 = 200
K = 200
TILES = ((0, 126), (126, 42))   # (pair offset, pair count) per partition tile


def build_nc():
    import concourse.bacc as bacc
    import concourse.bass as bass
    import concourse.mybir as mybir
    from concourse.bass import IndirectOffsetOnAxis
    from concourse.tile import TileContext

    f32 = mybir.dt.float32
    u32 = mybir.dt.uint32
    u8 = mybir.dt.uint8
    Alu = mybir.AluOpType
    Act = mybir.ActivationFunctionType
    Ax = mybir.AxisListType

    nc = bacc.Bacc()

    # HW indirect DMA semantics: ONE offset per partition per instruction
    # (multi-element offset APs execute but gather garbage). Emit K
    # instructions; they pipeline through the SWDGE queues at ~1us each.
    def indirect_blocks(cnt, *, out=None, in_=None, off=None, scatter=False):
        for sg in range(K):
            if scatter:
                nc.gpsimd.indirect_dma_start(
                    out=out, out_offset=IndirectOffsetOnAxis(
                        ap=off[:, sg:sg + 1], axis=0),
                    in_=in_[:, sg, :], in_offset=None,
                )
            else:
                nc.gpsimd.indirect_dma_start(
                    out=out[:, sg, :], out_offset=None,
                    in_=in_, in_offset=IndirectOffsetOnAxis(
                        ap=off[:, sg:sg + 1], axis=0),
                )

    def TT(out, in0, in1, op):
        # TensorTensor's ISA struct can't encode multiple sync waits (codegen
        # "Too many sync wait commands"); TensorScalarPtr can, so emit every
        # tensor-tensor op as (in0 bypass 0.0) op in1.
        nc.vector.scalar_tensor_tensor(
            out=out, in0=in0, scalar=0.0, in1=in1, op0=Alu.bypass, op1=op,
        )

    loc1 = nc.declare_dram_parameter("loc1", [IMG, NPAD, 4], f32, isOutput=False)
    loc2 = nc.declare_dram_parameter("loc2", [IMG, NPAD, 4], f32, isOutput=False)
    conf1 = nc.declare_dram_parameter("conf1", [IMG, NPAD, C], f32, isOutput=False)
    conf2 = nc.declare_dram_parameter("conf2", [IMG, NPAD, C], f32, isOutput=False)
    dbox = nc.declare_dram_parameter("dbox", [NPAD, 4], f32, isOutput=False)
    # aux[pair] = (img*W2 base row into box table, 1 if class==0 else 0)
    aux = nc.declare_dram_parameter("aux", [PAIRS, 2], u32, isOutput=False)
    outp = nc.declare_dram_parameter("out", [PAIRS * 201, 5], f32, isOutput=True)

    scoresD = nc.dram_tensor("scoresD", [PAIRS * NCHUNK, CHUNK], f32)
    boxesD = nc.dram_tensor("boxesD", [IMG * W2, 4], f32)
    cidsD = nc.dram_tensor("cidsD", [PAIRS * K, 1], u32)

    with TileContext(nc) as tc:
        with tc.tile_pool(name="Mpool", bufs=1) as mp:
            M_tiles = [
                mp.tile([cnt, NCHUNK], f32, tag=f"M{ti}", name=f"M{ti}")
                for ti, (off, cnt) in enumerate(TILES)
            ]
            # ---------------- phase 1: scores + boxes -----------------------
            with (
                tc.tile_pool(name="persist", bufs=1) as pp,
                tc.tile_pool(name="work", bufs=1) as wp,
            ):
                SA = pp.tile([126, W2], f32, tag="SA")
                SB = pp.tile([42, W2], f32, tag="SB")
                S_tiles = (SA, SB)

                dbox_t = pp.tile([128, SLOTS, 4], f32, tag="dbox")
                nc.sync.dma_start(
                    out=dbox_t[:, :, :],
                    in_=dbox.rearrange("(p s) c -> p s c", s=SLOTS),
                )

                for img in range(IMG):
                    for v, (locp, confp) in enumerate(
                        ((loc1, conf1), (loc2, conf2))
                    ):
                        # ---- decode ----
                        loc_t = wp.tile([128, SLOTS, 4], f32, tag="loc")
                        nc.sync.dma_start(
                            out=loc_t[:, :, :],
                            in_=locp[img].rearrange("(p s) c -> p s c", s=SLOTS),
                        )
                        box_t = wp.tile([128, SLOTS, 4], f32, tag="box")
                        wh_t = wp.tile([128, SLOTS, 2], f32, tag="wh")
                        t1_t = wp.tile([128, SLOTS, 2], f32, tag="dtmp")
                        # wh = dbox_wh * exp(0.2*loc_wh)
                        nc.scalar.activation(
                            wh_t[:, :, :], loc_t[:, :, 2:4], Act.Exp, scale=0.2
                        )
                        TT(
                            out=wh_t[:, :, :], in0=wh_t[:, :, :],
                            in1=dbox_t[:, :, 2:4], op=Alu.mult,
                        )
                        # cxy = dbox_xy + (loc_xy*0.1)*dbox_xy
                        nc.vector.tensor_scalar_mul(
                            t1_t[:, :, :], loc_t[:, :, :2], 0.1
                        )
                        TT(
                            out=t1_t[:, :, :], in0=t1_t[:, :, :],
                            in1=dbox_t[:, :, :2], op=Alu.mult,
                        )
                        TT(
                            out=t1_t[:, :, :], in0=t1_t[:, :, :],
                            in1=dbox_t[:, :, :2], op=Alu.add,
                        )
                        # mn = cxy - 0.5*wh ; mx = mn + wh
                        nc.vector.tensor_scalar_mul(
                            box_t[:, :, 2:4], wh_t[:, :, :], 0.5
                        )
                        TT(
                            out=box_t[:, :, 0:2], in0=t1_t[:, :, :],
                            in1=box_t[:, :, 2:4], op=Alu.subtract,
                        )
                        TT(
                            out=box_t[:, :, 2:4], in0=box_t[:, :, 0:2],
                            in1=wh_t[:, :, :], op=Alu.add,
                        )
                        if v == 1:
                            # flip: x1' = 1-x2, x2' = 1-x1
                            fx_t = wp.tile([128, SLOTS, 2], f32, tag="fx")
                            nc.vector.tensor_scalar(
                                fx_t[:, :, 0:1], box_t[:, :, 2:3], -1.0, 1.0,
                                op0=Alu.mult, op1=Alu.add,
                            )
                            nc.vector.tensor_scalar(
                                fx_t[:, :, 1:2], box_t[:, :, 0:1], -1.0, 1.0,
                                op0=Alu.mult, op1=Alu.add,
                            )
                            nc.vector.tensor_copy(
                                box_t[:, :, 0:1], fx_t[:, :, 0:1]
                            )
                            nc.vector.tensor_copy(
                                box_t[:, :, 2:3], fx_t[:, :, 1:2]
                            )
                        # boxesD row sigma = v*NPAD + pg*(SLOTS*32) + s*32 + l
                        for pg in range(4):
                            base = img * W2 + v * NPAD + pg * (SLOTS * 32)
                            nc.sync.dma_start(
                                out=boxesD[base:base + SLOTS * 32, :].rearrange(
                                    "(s l) c -> l s c", l=32
                                ),
                                in_=box_t[pg * 32:(pg + 1) * 32, :, :],
                            )

                        # ---- softmax (prior-major) ----
                        cf_t = wp.tile([128, SLOTS, C], f32, tag="cf")
                        nc.sync.dma_start(
                            out=cf_t[:, :, :],
                            in_=confp[img].rearrange("(p s) c -> p s c", s=SLOTS),
                        )
                        mx_t = wp.tile([128, SLOTS], f32, tag="mx")
                        nc.vector.tensor_reduce(
                            out=mx_t[:, :], in_=cf_t[:, :, :], axis=Ax.X,
                            op=Alu.max,
                        )
                        TT(
                            out=cf_t[:, :, :], in0=cf_t[:, :, :],
                            in1=mx_t[:, :, None].to_broadcast([128, SLOTS, C]),
                            op=Alu.subtract,
                        )
                        pr_t = wp.tile([128, SLOTS, 32], f32, tag="pr")
                        nc.vector.memset(pr_t[:, :, C:], 0.0)
                        nc.scalar.activation(
                            pr_t[:, :, :C], cf_t[:, :, :], Act.Exp
                        )
                        sm_t = wp.tile([128, SLOTS], f32, tag="sm")
                        nc.vector.tensor_reduce(
                            out=sm_t[:, :], in_=pr_t[:, :, :C], axis=Ax.X,
                            op=Alu.add,
                        )
                        nc.vector.reciprocal(sm_t[:, :], sm_t[:, :])
                        TT(
                            out=pr_t[:, :, :C], in0=pr_t[:, :, :C],
                            in1=sm_t[:, :, None].to_broadcast([128, SLOTS, C]),
                            op=Alu.mult,
                        )
                        # ---- 32x32 block transpose ----
                        tr_t = wp.tile([128, SLOTS, 32], f32, tag="tr")
                        nc.vector.transpose(
                            out=tr_t[:, :, :].rearrange("p s c -> p (s c)"),
                            in_=pr_t[:, :, :].rearrange("p s c -> p (s c)"),
                        )
                        # ---- SBUF->SBUF DMA into pair-major rows ----
                        if img < 6:
                            dst, row0 = SA, img * C
                        else:
                            dst, row0 = SB, (img - 6) * C
                        for pg in range(4):
                            nc.sync.dma_start(
                                out=dst[row0:row0 + C,
                                        v * NPAD + pg * (SLOTS * 32):
                                        v * NPAD + (pg + 1) * (SLOTS * 32)],
                                in_=tr_t[pg * 32: pg * 32 + C, :, :].rearrange(
                                    "c s l -> c (s l)"
                                ),
                            )

                # big copy of pair-major scores to DRAM + chunk max
                for (off, cnt), st, M_t in zip(TILES, S_tiles, M_tiles):
                    nc.sync.dma_start(
                        out=scoresD[off * NCHUNK:(off + cnt) * NCHUNK, :],
                        in_=st[:, :].rearrange("p (n k) -> p n k", k=CHUNK),
                    )
                    nc.vector.tensor_reduce(
                        out=M_t[:, :],
                        in_=st[:, :].rearrange("p (n k) -> p n k", k=CHUNK),
                        axis=Ax.X, op=Alu.max,
                    )
            # persist pool (SA/SB) freed here

            # zero the output (scatter only writes kept rows)
            with tc.tile_pool(name="zero", bufs=1) as zp:
                z_t = zp.tile([128, 201 * 5], f32, tag="z")
                nc.vector.memset(z_t[:, :], 0.0)
                for off, cnt in TILES:
                    nc.sync.dma_start(
                        out=outp[off * 201:(off + cnt) * 201, :].rearrange(
                            "(p r) c -> p (r c)", r=201),
                        in_=z_t[:cnt, :],
                    )

            # ---------------- phase 2: per-tile selection + NMS -------------
            with tc.tile_pool(name="sel", bufs=1) as sp:
              for ti, ((off, cnt), M_t) in enumerate(zip(TILES, M_tiles)):
                if True:
                    cids_t = sp.tile([cnt, K], u32, tag=f"cid{ti}")
                    v8 = sp.tile([cnt, 8], f32, tag=f"v8{ti}")
                    for r in range(ROUNDS):
                        nc.vector.max(out=v8[:, :], in_=M_t[:, :])
                        nc.vector.max_index(
                            out=cids_t[:, 8 * r:8 * r + 8],
                            in_max=v8[:, :], in_values=M_t[:, :],
                        )
                        nc.vector.match_replace(
                            out=M_t[:, :], in_to_replace=v8[:, :],
                            in_values=M_t[:, :], imm_value=-1.0,
                        )
                    nc.sync.dma_start(
                        out=cidsD[off * K:(off + cnt) * K, :],
                        in_=cids_t[:, :, None],
                    )
                    # gather the 200 selected chunks into the pool
                    base_t = sp.tile([cnt, 1], u32, tag=f"ba{ti}")
                    nc.gpsimd.iota(
                        base_t[:, :], pattern=[[0, 1]],
                        base=off * NCHUNK, channel_multiplier=NCHUNK,
                    )
                    gidx_t = sp.tile([cnt, K], u32, tag=f"gi{ti}")
                    TT(
                        out=gidx_t[:, :], in0=cids_t[:, :],
                        in1=base_t[:, :].to_broadcast([cnt, K]), op=Alu.add,
                    )
                    pool_t = sp.tile([cnt, K, CHUNK], f32, tag=f"po{ti}")
                    indirect_blocks(
                        cnt, out=pool_t[:, :, :], in_=scoresD[:, :],
                        off=gidx_t,
                    )
                    # exact sorted top-200 from the pool
                    sorted_t = sp.tile([cnt, K], f32, tag=f"so{ti}")
                    ps_t = sp.tile([cnt, K], u32, tag=f"ps{ti}")
                    poolf = pool_t[:, :, :].rearrange("p n k -> p (n k)")
                    for r in range(ROUNDS):
                        nc.vector.max(
                            out=sorted_t[:, 8 * r:8 * r + 8], in_=poolf
                        )
                        nc.vector.max_index(
                            out=ps_t[:, 8 * r:8 * r + 8],
                            in_max=sorted_t[:, 8 * r:8 * r + 8],
                            in_values=poolf,
                        )
                        nc.vector.match_replace(
                            out=poolf,
                            in_to_replace=sorted_t[:, 8 * r:8 * r + 8],
                            in_values=poolf, imm_value=-1.0,
                        )
                    # pool slot -> chunk id (DRAM bounce) -> sigma column
                    sh_t = sp.tile([cnt, K], u32, tag=f"sh{ti}")
                    nc.vector.tensor_scalar(
                        sh_t[:, :], ps_t[:, :], 3, None,
                        op0=Alu.logical_shift_right,
                    )
                    b2_t = sp.tile([cnt, 1], u32, tag=f"b2{ti}")
                    nc.gpsimd.iota(
                        b2_t[:, :], pattern=[[0, 1]],
                        base=off * K, channel_multiplier=K,
                    )
                    g2_t = sp.tile([cnt, K], u32, tag=f"g2{ti}")
                    TT(
                        out=g2_t[:, :], in0=sh_t[:, :],
                        in1=b2_t[:, :].to_broadcast([cnt, K]), op=Alu.add,
                    )
                    csel_t = sp.tile([cnt, K, 1], u32, tag=f"cs{ti}")
                    indirect_blocks(
                        cnt, out=csel_t[:, :, :], in_=cidsD[:, :], off=g2_t,
                    )
                    # sigma = cid*8 + (ps - (ps>>3)<<3)
                    lane_t = sp.tile([cnt, K], u32, tag=f"la{ti}")
                    nc.vector.tensor_scalar(
                        lane_t[:, :], sh_t[:, :], 3, None,
                        op0=Alu.logical_shift_left,
                    )
                    TT(
                        out=lane_t[:, :], in0=ps_t[:, :], in1=lane_t[:, :],
                        op=Alu.subtract,
                    )
                    sig_t = sp.tile([cnt, K], u32, tag=f"sg{ti}")
                    nc.vector.tensor_scalar(
                        sig_t[:, :], csel_t[:, :, 0], 3, None,
                        op0=Alu.logical_shift_left,
                    )
                    TT(
                        out=sig_t[:, :], in0=sig_t[:, :], in1=lane_t[:, :],
                        op=Alu.add,
                    )
                    # box row = sigma + img*W2 (aux col 0)
                    aux_t = sp.tile([cnt, 2], u32, tag=f"ax{ti}")
                    nc.sync.dma_start(
                        out=aux_t[:, :], in_=aux[off:off + cnt, :]
                    )
                    TT(
                        out=sig_t[:, :], in0=sig_t[:, :],
                        in1=aux_t[:, 0:1].to_broadcast([cnt, K]), op=Alu.add,
                    )
                    bx_t = sp.tile([cnt, K, 4], f32, tag=f"bx{ti}")
                    indirect_blocks(
                        cnt, out=bx_t[:, :, :], in_=boxesD[:, :], off=sig_t,
                    )

                    # ---- IoU mask: S[i,j] = 1.45*inter - 0.45*(ai+aj) > 0 ----
                    ar_t = sp.tile([cnt, K], f32, tag=f"ar{ti}")
                    w0_t = sp.tile([cnt, K], f32, tag=f"w0{ti}")
                    TT(
                        out=w0_t[:, :], in0=bx_t[:, :, 2], in1=bx_t[:, :, 0],
                        op=Alu.subtract,
                    )
                    TT(
                        out=ar_t[:, :], in0=bx_t[:, :, 3], in1=bx_t[:, :, 1],
                        op=Alu.subtract,
                    )
                    TT(
                        out=ar_t[:, :], in0=ar_t[:, :], in1=w0_t[:, :],
                        op=Alu.mult,
                    )
                    Sm_t = sp.tile([cnt, K, K], u8, tag=f"Sm{ti}")
                    xa = sp.tile([cnt, 8, K], f32, tag="xa", name="xa")
                    xb = sp.tile([cnt, 8, K], f32, tag="xb", name="xb")
                    xc = sp.tile([cnt, 8, K], f32, tag="xc", name="xc")
                    for bi in range(K // 8):
                        r0 = 8 * bi
                        rows = bx_t[:, r0:r0 + 8, :]
                        TT(
                            out=xa[:, :, :],
                            in0=rows[:, :, 0:1].to_broadcast([cnt, 8, K]),
                            in1=bx_t[:, None, :, 0].to_broadcast([cnt, 8, K]),
                            op=Alu.max,
                        )
                        TT(
                            out=xb[:, :, :],
                            in0=rows[:, :, 2:3].to_broadcast([cnt, 8, K]),
                            in1=bx_t[:, None, :, 2].to_broadcast([cnt, 8, K]),
                            op=Alu.min,
                        )
                        TT(
                            out=xa[:, :, :], in0=xb[:, :, :], in1=xa[:, :, :],
                            op=Alu.subtract,
                        )
                        nc.vector.tensor_scalar_max(
                            xa[:, :, :], xa[:, :, :], 0.0
                        )
                        TT(
                            out=xc[:, :, :],
                            in0=rows[:, :, 1:2].to_broadcast([cnt, 8, K]),
                            in1=bx_t[:, None, :, 1].to_broadcast([cnt, 8, K]),
                            op=Alu.max,
                        )
                        TT(
                            out=xb[:, :, :],
                            in0=rows[:, :, 3:4].to_broadcast([cnt, 8, K]),
                            in1=bx_t[:, None, :, 3].to_broadcast([cnt, 8, K]),
                            op=Alu.min,
                        )
                        TT(
                            out=xb[:, :, :], in0=xb[:, :, :], in1=xc[:, :, :],
                            op=Alu.subtract,
                        )
                        nc.vector.tensor_scalar_max(
                            xb[:, :, :], xb[:, :, :], 0.0
                        )
                        TT(
                            out=xa[:, :, :], in0=xa[:, :, :], in1=xb[:, :, :],
                            op=Alu.mult,
                        )
                        TT(
                            out=xb[:, :, :],
                            in0=ar_t[:, r0:r0 + 8, None].to_broadcast(
                                [cnt, 8, K]),
                            in1=ar_t[:, None, :].to_broadcast([cnt, 8, K]),
                            op=Alu.add,
                        )
                        nc.vector.tensor_scalar_mul(
                            xa[:, :, :], xa[:, :, :], 1.45
                        )
                        nc.vector.scalar_tensor_tensor(
                            out=xa[:, :, :], in0=xb[:, :, :], scalar=-0.45,
                            in1=xa[:, :, :], op0=Alu.mult, op1=Alu.add,
                        )
                        nc.vector.tensor_scalar(
                            Sm_t[:, r0:r0 + 8, :], xa[:, :, :], 0.0, None,
                            op0=Alu.is_gt,
                        )

                    # ---- greedy NMS scan ----
                    keep_t = sp.tile([cnt, K], u8, tag=f"ke{ti}")
                    nc.vector.memset(keep_t[:, :], 0)
                    tmp_t = sp.tile([cnt, K], u8, tag=f"tm{ti}")
                    sup_t = sp.tile([cnt, 1], u8, tag=f"su{ti}")
                    for i in range(K):
                        TT(
                            out=tmp_t[:, :], in0=Sm_t[:, i, :],
                            in1=keep_t[:, :], op=Alu.mult,
                        )
                        nc.vector.tensor_reduce(
                            out=sup_t[:, :], in_=tmp_t[:, :], axis=Ax.X,
                            op=Alu.max,
                        )
                        nc.vector.tensor_scalar(
                            keep_t[:, i:i + 1], sup_t[:, :], 0, None,
                            op0=Alu.is_equal,
                        )

                    # ---- output scatter ----
                    keepf_t = sp.tile([cnt, K], f32, tag=f"kf{ti}")
                    nc.vector.tensor_copy(keepf_t[:, :], keep_t[:, :])
                    pos_t = sp.tile([cnt, K], f32, tag=f"pf{ti}")
                    nc.vector.tensor_tensor_scan(
                        out=pos_t[:, :], data0=keepf_t[:, :],
                        data1=keepf_t[:, :], initial=-1.0,
                        op0=Alu.add, op1=Alu.bypass,
                    )
                    posx_t = sp.tile([cnt, K], f32, tag=f"px{ti}")
                    nc.vector.memset(posx_t[:, :], float(K))
                    nc.vector.copy_predicated(
                        posx_t[:, :], keep_t[:, :], pos_t[:, :]
                    )
                    posu_t = sp.tile([cnt, K], u32, tag=f"pu{ti}")
                    nc.vector.tensor_copy(posu_t[:, :], posx_t[:, :])
                    # class-0 pairs always go to the trash row
                    cls0_t = sp.tile([cnt, K], u32, tag=f"c0{ti}")
                    nc.vector.tensor_scalar(
                        cls0_t[:, :],
                        aux_t[:, 1:2].to_broadcast([cnt, K]), K, None,
                        op0=Alu.mult,
                    )
                    TT(
                        out=posu_t[:, :], in0=posu_t[:, :], in1=cls0_t[:, :],
                        op=Alu.max,
                    )
                    b3_t = sp.tile([cnt, 1], u32, tag=f"b3{ti}")
                    nc.gpsimd.iota(
                        b3_t[:, :], pattern=[[0, 1]],
                        base=off * 201, channel_multiplier=201,
                    )
                    TT(
                        out=posu_t[:, :], in0=posu_t[:, :],
                        in1=b3_t[:, :].to_broadcast([cnt, K]), op=Alu.add,
                    )
                    row_t = sp.tile([cnt, K, 5], f32, tag=f"ro{ti}")
                    nc.vector.tensor_copy(row_t[:, :, 0], sorted_t[:, :])
                    nc.vector.tensor_copy(row_t[:, :, 1:5], bx_t[:, :, :])
                    indirect_blocks(
                        cnt, out=outp[:, :], in_=row_t, off=posu_t,
                        scatter=True,
                    )
    nc.compile()
    return nc


def _prep_core_inputs(loc_b, conf_b, loc2_b, conf2_b, dbox):
    """Pad per-core inputs to NPAD priors; build aux table."""
    pad = NPAD - N
    locp = np.pad(loc_b, ((0, 0), (0, pad), (0, 0)))
    loc2p = np.pad(loc2_b, ((0, 0), (0, pad), (0, 0)))
    cpad = np.zeros((conf_b.shape[0], pad, C), np.float32)
    cpad[:, :, 0] = 40.0
    cpad[:, :, 1:] = -40.0
    confp = np.concatenate([conf_b, cpad], axis=1)
    conf2p = np.concatenate([conf2_b, cpad], axis=1)
    dpad = np.zeros((pad, 4), np.float32)
    dpad[:, 2:] = 1e-3
    dboxp = np.concatenate([dbox, dpad], axis=0)
    aux = np.zeros((PAIRS, 2), np.uint32)
    for p in range(PAIRS):
        aux[p, 0] = (p // C) * W2
        aux[p, 1] = 1 if (p % C) == 0 else 0
    return {
        "loc1": np.ascontiguousarray(locp, np.float32),
        "loc2": np.ascontiguousarray(loc2p, np.float32),
        "conf1": np.ascontiguousarray(confp, np.float32),
        "conf2": np.ascontiguousarray(conf2p, np.float32),
        "dbox": np.ascontiguousarray(dboxp, np.float32),
        "aux": aux,
    }


def kernel(loc_data, conf_data, loc_data2, conf_data2, dbox_list):
    from concourse.bass_utils import run_bass_kernel_spmd

    loc_data = np.asarray(loc_data, np.float32)
    conf_data = np.asarray(conf_data, np.float32)
    loc_data2 = np.asarray(loc_data2, np.float32)
    conf_data2 = np.asarray(conf_data2, np.float32)
    dbox_list = np.asarray(dbox_list, np.float32)

    nc = build_nc()
    in_maps = []
    for k in range(8):
        sl = slice(k * IMG, (k + 1) * IMG)
        in_maps.append(
            _prep_core_inputs(
                loc_data[sl], conf_data[sl], loc_data2[sl], conf_data2[sl],
                dbox_list,
            )
        )
    res = run_bass_kernel_spmd(nc, in_maps, list(range(8))).results
    outs = []
    for k in range(8):
        o = np.asarray(res[k]["out"]).reshape(PAIRS, 201, 5)[:, :K, :]
        outs.append(o.reshape(IMG, C, K, 5))
    return np.concatenate(outs, axis=0)



# revision 15
# speedup vs baseline: 12.7392x; 1.5740x over previous
"""SSD detection post-processing (decode + softmax + per-class top-200 + NMS,
TTA-flip merge) as a Bass/Tile kernel for 8 Trainium2 NeuronCores.

Sharding: pure data parallel over the batch dim — core k handles images
8k..8k+7 (= 168 (image,class) pairs per core).

Per-core pipeline (all on device):
  1. decode both views' boxes (flip view 2), store to a DRAM box table
  2. softmax probs in prior-major layout; 32x32 stream-transpose + SBUF-SBUF
     DMA reassembly into pair-major score rows [pair, 17664]
  3. chunk-max (L=8) -> M [pair, 2208]; 25 rounds of max8/max_index/
     match_replace extract the 200 largest chunk maxes (provably a superset
     of the chunks holding the global top-200: if >200 chunks had max above
     the 201st value, there would be >200 elements above it)
  4. indirect-DMA gather of those chunks -> pool [pair, 1600]; 25 more
     extraction rounds give the exact sorted top-200 + pool slots
  5. map pool slots -> chunk ids -> score-column index; indirect-gather boxes
  6. upper-triangle IoU mask (iou > 0.45 as 1.45*inter - 0.45*(ai+aj) > 0);
     only Sm[i, k>i] is ever read by the scan, so the lower half is skipped
  7. greedy NMS via a running suppression vector: keep[i] = (supv[i]==0);
     supv[k>i] max= keep[i]*Sm[i, k>i] — two fused DVE ops per step
  8. compacted survivor rows scattered into the output via indirect DMA

The 168 pairs exceed the 128 SBUF partitions, so phase 2 runs as two
partition tiles (126 + 42 pairs). Their instruction streams are interleaved
op-by-op: the engines' in-order queues then always have an independent op
from the other tile to issue while a dependent chain waits, which hides
most dependency latency.
"""

import numpy as np

B = 64
N = 8732
C = 21
NPAD = 8832            # priors padded to 69*128
SLOTS = NPAD // 128    # 69
W2 = 2 * NPAD          # 17664 score columns per pair
IMG = 8                # images per core
PAIRS = IMG * C        # 168 pairs per core
CHUNK = 8
NCHUNK = W2 // CHUNK   # 2208
ROUNDS = 25            # 25*8 = 200
K = 200
RIOU = 8               # IoU block rows
TILES = ((0, 126), (126, 42))   # (pair offset, pair count) per partition tile


def build_nc(level=99):
    # level: phase-truncation for perf diagnosis (99 = full kernel).
    # 2=+M extraction; 3=+pool gather+sort; 4=+index mapping+box gather;
    # 5=+IoU mask; 6=+NMS scan; 7/99=+scatter.
    import concourse.bacc as bacc
    import concourse.bass as bass
    import concourse.mybir as mybir
    from concourse.bass import IndirectOffsetOnAxis
    from concourse.tile import TileContext

    f32 = mybir.dt.float32
    u32 = mybir.dt.uint32
    u8 = mybir.dt.uint8
    Alu = mybir.AluOpType
    Act = mybir.ActivationFunctionType
    Ax = mybir.AxisListType

    nc = bacc.Bacc()

    def TT(out, in0, in1, op):
        # TensorTensor's ISA struct can't encode multiple sync waits (codegen
        # "Too many sync wait commands"); TensorScalarPtr can, so emit every
        # tensor-tensor op as (in0 bypass 0.0) op in1.
        nc.vector.scalar_tensor_tensor(
            out=out, in0=in0, scalar=0.0, in1=in1, op0=Alu.bypass, op1=op,
        )

    loc1 = nc.declare_dram_parameter("loc1", [IMG, NPAD, 4], f32, isOutput=False)
    loc2 = nc.declare_dram_parameter("loc2", [IMG, NPAD, 4], f32, isOutput=False)
    conf1 = nc.declare_dram_parameter("conf1", [IMG, NPAD, C], f32, isOutput=False)
    conf2 = nc.declare_dram_parameter("conf2", [IMG, NPAD, C], f32, isOutput=False)
    dbox = nc.declare_dram_parameter("dbox", [NPAD, 4], f32, isOutput=False)
    # aux[pair] = (img*W2 base row into box table, 1 if class==0 else 0)
    aux = nc.declare_dram_parameter("aux", [PAIRS, 2], u32, isOutput=False)
    outp = nc.declare_dram_parameter("out", [PAIRS * 201, 5], f32, isOutput=True)

    scoresD = nc.dram_tensor("scoresD", [PAIRS * NCHUNK, CHUNK], f32)
    boxesD = nc.dram_tensor("boxesD", [IMG * W2, 4], f32)
    cidsD = nc.dram_tensor("cidsD", [PAIRS * K, 1], u32)

    with TileContext(nc) as tc:
        with tc.tile_pool(name="Mpool", bufs=1) as mp:
            M_tiles = [
                mp.tile([cnt, NCHUNK], f32, tag=f"M{ti}", name=f"M{ti}")
                for ti, (off, cnt) in enumerate(TILES)
            ]
            # ---------------- phase 1: scores + boxes -----------------------
            with (
                tc.tile_pool(name="persist", bufs=1) as pp,
                tc.tile_pool(name="work", bufs=1) as wp,
            ):
                SA = pp.tile([126, W2], f32, tag="SA")
                SB = pp.tile([42, W2], f32, tag="SB")
                S_tiles = (SA, SB)

                dbox_t = pp.tile([128, SLOTS, 4], f32, tag="dbox")
                nc.sync.dma_start(
                    out=dbox_t[:, :, :],
                    in_=dbox.rearrange("(p s) c -> p s c", s=SLOTS),
                )

                for img in range(IMG):
                    for v, (locp, confp) in enumerate(
                        ((loc1, conf1), (loc2, conf2))
                    ):
                        # ---- decode ----
                        loc_t = wp.tile([128, SLOTS, 4], f32, tag="loc")
                        nc.sync.dma_start(
                            out=loc_t[:, :, :],
                            in_=locp[img].rearrange("(p s) c -> p s c", s=SLOTS),
                        )
                        box_t = wp.tile([128, SLOTS, 4], f32, tag="box")
                        wh_t = wp.tile([128, SLOTS, 2], f32, tag="wh")
                        t1_t = wp.tile([128, SLOTS, 2], f32, tag="dtmp")
                        # wh = dbox_wh * exp(0.2*loc_wh)
                        nc.scalar.activation(
                            wh_t[:, :, :], loc_t[:, :, 2:4], Act.Exp, scale=0.2
                        )
                        TT(
                            out=wh_t[:, :, :], in0=wh_t[:, :, :],
                            in1=dbox_t[:, :, 2:4], op=Alu.mult,
                        )
                        # cxy = dbox_xy + (loc_xy*0.1)*dbox_xy
                        nc.vector.tensor_scalar_mul(
                            t1_t[:, :, :], loc_t[:, :, :2], 0.1
                        )
                        TT(
                            out=t1_t[:, :, :], in0=t1_t[:, :, :],
                            in1=dbox_t[:, :, :2], op=Alu.mult,
                        )
                        TT(
                            out=t1_t[:, :, :], in0=t1_t[:, :, :],
                            in1=dbox_t[:, :, :2], op=Alu.add,
                        )
                        # mn = cxy - 0.5*wh ; mx = mn + wh
                        nc.vector.tensor_scalar_mul(
                            box_t[:, :, 2:4], wh_t[:, :, :], 0.5
                        )
                        TT(
                            out=box_t[:, :, 0:2], in0=t1_t[:, :, :],
                            in1=box_t[:, :, 2:4], op=Alu.subtract,
                        )
                        TT(
                            out=box_t[:, :, 2:4], in0=box_t[:, :, 0:2],
                            in1=wh_t[:, :, :], op=Alu.add,
                        )
                        if v == 1:
                            # flip: x1' = 1-x2, x2' = 1-x1
                            fx_t = wp.tile([128, SLOTS, 2], f32, tag="fx")
                            nc.vector.tensor_scalar(
                                fx_t[:, :, 0:1], box_t[:, :, 2:3], -1.0, 1.0,
                                op0=Alu.mult, op1=Alu.add,
                            )
                            nc.vector.tensor_scalar(
                                fx_t[:, :, 1:2], box_t[:, :, 0:1], -1.0, 1.0,
                                op0=Alu.mult, op1=Alu.add,
                            )
                            nc.vector.tensor_copy(
                                box_t[:, :, 0:1], fx_t[:, :, 0:1]
                            )
                            nc.vector.tensor_copy(
                                box_t[:, :, 2:3], fx_t[:, :, 1:2]
                            )
                        # boxesD row sigma = v*NPAD + pg*(SLOTS*32) + s*32 + l
                        for pg in range(4):
                            base = img * W2 + v * NPAD + pg * (SLOTS * 32)
                            nc.sync.dma_start(
                                out=boxesD[base:base + SLOTS * 32, :].rearrange(
                                    "(s l) c -> l s c", l=32
                                ),
                                in_=box_t[pg * 32:(pg + 1) * 32, :, :],
                            )

                        # ---- softmax (prior-major) ----
                        cf_t = wp.tile([128, SLOTS, C], f32, tag="cf")
                        nc.sync.dma_start(
                            out=cf_t[:, :, :],
                            in_=confp[img].rearrange("(p s) c -> p s c", s=SLOTS),
                        )
                        mx_t = wp.tile([128, SLOTS], f32, tag="mx")
                        nc.vector.tensor_reduce(
                            out=mx_t[:, :], in_=cf_t[:, :, :], axis=Ax.X,
                            op=Alu.max,
                        )
                        TT(
                            out=cf_t[:, :, :], in0=cf_t[:, :, :],
                            in1=mx_t[:, :, None].to_broadcast([128, SLOTS, C]),
                            op=Alu.subtract,
                        )
                        pr_t = wp.tile([128, SLOTS, 32], f32, tag="pr")
                        nc.vector.memset(pr_t[:, :, C:], 0.0)
                        nc.scalar.activation(
                            pr_t[:, :, :C], cf_t[:, :, :], Act.Exp
                        )
                        sm_t = wp.tile([128, SLOTS], f32, tag="sm")
                        nc.vector.tensor_reduce(
                            out=sm_t[:, :], in_=pr_t[:, :, :C], axis=Ax.X,
                            op=Alu.add,
                        )
                        nc.vector.reciprocal(sm_t[:, :], sm_t[:, :])
                        TT(
                            out=pr_t[:, :, :C], in0=pr_t[:, :, :C],
                            in1=sm_t[:, :, None].to_broadcast([128, SLOTS, C]),
                            op=Alu.mult,
                        )
                        # ---- 32x32 block transpose ----
                        tr_t = wp.tile([128, SLOTS, 32], f32, tag="tr")
                        nc.vector.transpose(
                            out=tr_t[:, :, :].rearrange("p s c -> p (s c)"),
                            in_=pr_t[:, :, :].rearrange("p s c -> p (s c)"),
                        )
                        # ---- SBUF->SBUF DMA into pair-major rows ----
                        if img < 6:
                            dst, row0 = SA, img * C
                        else:
                            dst, row0 = SB, (img - 6) * C
                        for pg in range(4):
                            nc.sync.dma_start(
                                out=dst[row0:row0 + C,
                                        v * NPAD + pg * (SLOTS * 32):
                                        v * NPAD + (pg + 1) * (SLOTS * 32)],
                                in_=tr_t[pg * 32: pg * 32 + C, :, :].rearrange(
                                    "c s l -> c (s l)"
                                ),
                            )

                # big copy of pair-major scores to DRAM + chunk max
                for (off, cnt), st, M_t in zip(TILES, S_tiles, M_tiles):
                    nc.sync.dma_start(
                        out=scoresD[off * NCHUNK:(off + cnt) * NCHUNK, :],
                        in_=st[:, :].rearrange("p (n k) -> p n k", k=CHUNK),
                    )
                    nc.vector.tensor_reduce(
                        out=M_t[:, :],
                        in_=st[:, :].rearrange("p (n k) -> p n k", k=CHUNK),
                        axis=Ax.X, op=Alu.max,
                    )
            # persist pool (SA/SB) freed here

            # zero the output (scatter only writes kept rows)
            with tc.tile_pool(name="zero", bufs=1) as zp:
                z_t = zp.tile([128, 201 * 5], f32, tag="z")
                nc.vector.memset(z_t[:, :], 0.0)
                for off, cnt in TILES:
                    nc.sync.dma_start(
                        out=outp[off * 201:(off + cnt) * 201, :].rearrange(
                            "(p r) c -> p (r c)", r=201),
                        in_=z_t[:cnt, :],
                    )

            # -------- phase 2: selection + NMS, tiles interleaved -----------
            if level < 2:
                nc.compile()
                return nc
            with tc.tile_pool(name="sel", bufs=1) as sp:
                D = []   # per-tile working tiles
                for ti, (off, cnt) in enumerate(TILES):
                    d = {}
                    d["cids"] = sp.tile([cnt, K], u32, tag=f"cid{ti}", name=f"cid{ti}")
                    d["v8"] = sp.tile([cnt, 8], f32, tag=f"v8{ti}", name=f"v8{ti}")
                    D.append(d)

                # ---- top-200 chunk extraction (interleaved rounds) ----
                for r in range(ROUNDS):
                    for ti, (off, cnt) in enumerate(TILES):
                        M_t, d = M_tiles[ti], D[ti]
                        nc.vector.max(out=d["v8"][:, :], in_=M_t[:, :])
                        nc.vector.max_index(
                            out=d["cids"][:, 8 * r:8 * r + 8],
                            in_max=d["v8"][:, :], in_values=M_t[:, :],
                        )
                        nc.vector.match_replace(
                            out=M_t[:, :], in_to_replace=d["v8"][:, :],
                            in_values=M_t[:, :], imm_value=-1.0,
                        )
                if level < 3:
                    nc.compile()
                    return nc

                # ---- chunk-id table to DRAM + pool gather ----
                for ti, (off, cnt) in enumerate(TILES):
                    d = D[ti]
                    nc.sync.dma_start(
                        out=cidsD[off * K:(off + cnt) * K, :],
                        in_=d["cids"][:, :, None],
                    )
                    d["base"] = sp.tile([cnt, 1], u32, tag=f"ba{ti}", name=f"ba{ti}")
                    nc.gpsimd.iota(
                        d["base"][:, :], pattern=[[0, 1]],
                        base=off * NCHUNK, channel_multiplier=NCHUNK,
                    )
                    d["gidx"] = sp.tile([cnt, K], u32, tag=f"gi{ti}", name=f"gi{ti}")
                    TT(
                        out=d["gidx"][:, :], in0=d["cids"][:, :],
                        in1=d["base"][:, :].to_broadcast([cnt, K]), op=Alu.add,
                    )
                    d["pool"] = sp.tile([cnt, K, CHUNK], f32, tag=f"po{ti}", name=f"po{ti}")
                for sg in range(K):
                    for ti, (off, cnt) in enumerate(TILES):
                        d = D[ti]
                        nc.gpsimd.indirect_dma_start(
                            out=d["pool"][:, sg, :], out_offset=None,
                            in_=scoresD[:, :],
                            in_offset=IndirectOffsetOnAxis(
                                ap=d["gidx"][:, sg:sg + 1], axis=0),
                        )

                # ---- exact sorted top-200 from the pool (interleaved) ----
                for ti, (off, cnt) in enumerate(TILES):
                    d = D[ti]
                    d["sorted"] = sp.tile([cnt, K], f32, tag=f"so{ti}", name=f"so{ti}")
                    d["ps"] = sp.tile([cnt, K], u32, tag=f"ps{ti}", name=f"ps{ti}")
                for r in range(ROUNDS):
                    for ti, (off, cnt) in enumerate(TILES):
                        d = D[ti]
                        poolf = d["pool"][:, :, :].rearrange("p n k -> p (n k)")
                        nc.vector.max(
                            out=d["sorted"][:, 8 * r:8 * r + 8], in_=poolf
                        )
                        nc.vector.max_index(
                            out=d["ps"][:, 8 * r:8 * r + 8],
                            in_max=d["sorted"][:, 8 * r:8 * r + 8],
                            in_values=poolf,
                        )
                        nc.vector.match_replace(
                            out=poolf,
                            in_to_replace=d["sorted"][:, 8 * r:8 * r + 8],
                            in_values=poolf, imm_value=-1.0,
                        )
                if level < 4:
                    nc.compile()
                    return nc

                # ---- pool slot -> chunk id (DRAM bounce) -> sigma column ----
                for ti, (off, cnt) in enumerate(TILES):
                    d = D[ti]
                    d["sh"] = sp.tile([cnt, K], u32, tag=f"sh{ti}", name=f"sh{ti}")
                    nc.vector.tensor_scalar(
                        d["sh"][:, :], d["ps"][:, :], 3, None,
                        op0=Alu.logical_shift_right,
                    )
                    d["b2"] = sp.tile([cnt, 1], u32, tag=f"b2{ti}", name=f"b2{ti}")
                    nc.gpsimd.iota(
                        d["b2"][:, :], pattern=[[0, 1]],
                        base=off * K, channel_multiplier=K,
                    )
                    d["g2"] = sp.tile([cnt, K], u32, tag=f"g2{ti}", name=f"g2{ti}")
                    TT(
                        out=d["g2"][:, :], in0=d["sh"][:, :],
                        in1=d["b2"][:, :].to_broadcast([cnt, K]), op=Alu.add,
                    )
                    d["csel"] = sp.tile([cnt, K, 1], u32, tag=f"cs{ti}", name=f"cs{ti}")
                for sg in range(K):
                    for ti, (off, cnt) in enumerate(TILES):
                        d = D[ti]
                        nc.gpsimd.indirect_dma_start(
                            out=d["csel"][:, sg, :], out_offset=None,
                            in_=cidsD[:, :],
                            in_offset=IndirectOffsetOnAxis(
                                ap=d["g2"][:, sg:sg + 1], axis=0),
                        )
                # sigma = cid*8 + (ps - (ps>>3)<<3); box row = sigma + img*W2
                for ti, (off, cnt) in enumerate(TILES):
                    d = D[ti]
                    d["lane"] = sp.tile([cnt, K], u32, tag=f"la{ti}", name=f"la{ti}")
                    nc.vector.tensor_scalar(
                        d["lane"][:, :], d["sh"][:, :], 3, None,
                        op0=Alu.logical_shift_left,
                    )
                    TT(
                        out=d["lane"][:, :], in0=d["ps"][:, :],
                        in1=d["lane"][:, :], op=Alu.subtract,
                    )
                    d["sig"] = sp.tile([cnt, K], u32, tag=f"sg{ti}", name=f"sg{ti}")
                    nc.vector.tensor_scalar(
                        d["sig"][:, :], d["csel"][:, :, 0], 3, None,
                        op0=Alu.logical_shift_left,
                    )
                    TT(
                        out=d["sig"][:, :], in0=d["sig"][:, :],
                        in1=d["lane"][:, :], op=Alu.add,
                    )
                    d["aux"] = sp.tile([cnt, 2], u32, tag=f"ax{ti}", name=f"ax{ti}")
                    nc.sync.dma_start(
                        out=d["aux"][:, :], in_=aux[off:off + cnt, :]
                    )
                    TT(
                        out=d["sig"][:, :], in0=d["sig"][:, :],
                        in1=d["aux"][:, 0:1].to_broadcast([cnt, K]),
                        op=Alu.add,
                    )
                    d["bx"] = sp.tile([cnt, K, 4], f32, tag=f"bx{ti}", name=f"bx{ti}")
                for sg in range(K):
                    for ti, (off, cnt) in enumerate(TILES):
                        d = D[ti]
                        nc.gpsimd.indirect_dma_start(
                            out=d["bx"][:, sg, :], out_offset=None,
                            in_=boxesD[:, :],
                            in_offset=IndirectOffsetOnAxis(
                                ap=d["sig"][:, sg:sg + 1], axis=0),
                        )
                if level < 5:
                    nc.compile()
                    return nc

                # ---- upper-triangle IoU mask (interleaved blocks) ----
                # S[i,k] = 1.45*inter - 0.45*(ai+ak) > 0, computed only for
                # k >= r0 of each row block [r0, r0+RIOU) — the NMS scan
                # reads only k > i.
                for ti, (off, cnt) in enumerate(TILES):
                    d = D[ti]
                    d["ar"] = sp.tile([cnt, K], f32, tag=f"ar{ti}", name=f"ar{ti}")
                    d["w0"] = sp.tile([cnt, K], f32, tag=f"w0{ti}", name=f"w0{ti}")
                    TT(
                        out=d["w0"][:, :], in0=d["bx"][:, :, 2],
                        in1=d["bx"][:, :, 0], op=Alu.subtract,
                    )
                    TT(
                        out=d["ar"][:, :], in0=d["bx"][:, :, 3],
                        in1=d["bx"][:, :, 1], op=Alu.subtract,
                    )
                    TT(
                        out=d["ar"][:, :], in0=d["ar"][:, :],
                        in1=d["w0"][:, :], op=Alu.mult,
                    )
                    d["Sm"] = sp.tile([cnt, K, K], u8, tag=f"Sm{ti}", name=f"Sm{ti}")
                    d["xa"] = sp.tile([cnt, RIOU, K], f32, tag=f"xa{ti}", name=f"xa{ti}")
                    d["xb"] = sp.tile([cnt, RIOU, K], f32, tag=f"xb{ti}", name=f"xb{ti}")
                    d["xc"] = sp.tile([cnt, RIOU, K], f32, tag=f"xc{ti}", name=f"xc{ti}")
                for bi in range(K // RIOU):
                    r0 = RIOU * bi
                    W = K - r0
                    for ti, (off, cnt) in enumerate(TILES):
                        d = D[ti]
                        bx = d["bx"]
                        rows = bx[:, r0:r0 + RIOU, :]
                        sh3 = [cnt, RIOU, W]
                        xa = d["xa"][:, :, :W]
                        xb = d["xb"][:, :, :W]
                        xc = d["xc"][:, :, :W]
                        TT(
                            out=xc,
                            in0=rows[:, :, 1:2].to_broadcast(sh3),
                            in1=bx[:, None, r0:, 1].to_broadcast(sh3),
                            op=Alu.max,
                        )
                        TT(
                            out=xb,
                            in0=rows[:, :, 3:4].to_broadcast(sh3),
                            in1=bx[:, None, r0:, 3].to_broadcast(sh3),
                            op=Alu.min,
                        )
                        TT(out=xb, in0=xb, in1=xc, op=Alu.subtract)
                        nc.vector.tensor_scalar_max(xb, xb, 0.0)
                        TT(
                            out=xa,
                            in0=rows[:, :, 0:1].to_broadcast(sh3),
                            in1=bx[:, None, r0:, 0].to_broadcast(sh3),
                            op=Alu.max,
                        )
                        TT(
                            out=xc,
                            in0=rows[:, :, 2:3].to_broadcast(sh3),
                            in1=bx[:, None, r0:, 2].to_broadcast(sh3),
                            op=Alu.min,
                        )
                        TT(out=xa, in0=xc, in1=xa, op=Alu.subtract)
                        nc.vector.tensor_scalar_max(xa, xa, 0.0)
                        TT(out=xa, in0=xa, in1=xb, op=Alu.mult)
                        TT(
                            out=xb,
                            in0=d["ar"][:, r0:r0 + RIOU, None].to_broadcast(
                                sh3),
                            in1=d["ar"][:, None, r0:].to_broadcast(sh3),
                            op=Alu.add,
                        )
                        nc.vector.tensor_scalar_mul(xa, xa, 1.45)
                        nc.vector.scalar_tensor_tensor(
                            out=xa, in0=xb, scalar=-0.45, in1=xa,
                            op0=Alu.mult, op1=Alu.add,
                        )
                        nc.vector.tensor_scalar(
                            d["Sm"][:, r0:r0 + RIOU, r0:], xa, 0.0, None,
                            op0=Alu.is_gt,
                        )
                if level < 6:
                    nc.compile()
                    return nc

                # ---- greedy NMS scan (running suppression vector) ----
                for ti, (off, cnt) in enumerate(TILES):
                    d = D[ti]
                    d["keep"] = sp.tile([cnt, K], u8, tag=f"ke{ti}", name=f"ke{ti}")
                    d["supv"] = sp.tile([cnt, K], u8, tag=f"sv{ti}", name=f"sv{ti}")
                    nc.vector.memset(d["supv"][:, :], 0)
                for i in range(K):
                    for ti, (off, cnt) in enumerate(TILES):
                        d = D[ti]
                        nc.vector.tensor_scalar(
                            d["keep"][:, i:i + 1], d["supv"][:, i:i + 1],
                            0, None, op0=Alu.is_equal,
                        )
                        if i < K - 1:
                            nc.vector.scalar_tensor_tensor(
                                out=d["supv"][:, i + 1:],
                                in0=d["Sm"][:, i, i + 1:],
                                scalar=d["keep"][:, i:i + 1],
                                in1=d["supv"][:, i + 1:],
                                op0=Alu.mult, op1=Alu.max,
                            )
                if level < 7:
                    nc.compile()
                    return nc

                # ---- output scatter ----
                for ti, (off, cnt) in enumerate(TILES):
                    d = D[ti]
                    keepf = sp.tile([cnt, K], f32, tag=f"kf{ti}")
                    nc.vector.tensor_copy(keepf[:, :], d["keep"][:, :])
                    pos = sp.tile([cnt, K], f32, tag=f"pf{ti}")
                    nc.vector.tensor_tensor_scan(
                        out=pos[:, :], data0=keepf[:, :],
                        data1=keepf[:, :], initial=-1.0,
                        op0=Alu.add, op1=Alu.bypass,
                    )
                    posx = sp.tile([cnt, K], f32, tag=f"px{ti}")
                    nc.vector.memset(posx[:, :], float(K))
                    nc.vector.copy_predicated(
                        posx[:, :], d["keep"][:, :], pos[:, :]
                    )
                    posu = sp.tile([cnt, K], u32, tag=f"pu{ti}")
                    nc.vector.tensor_copy(posu[:, :], posx[:, :])
                    # class-0 pairs always go to the trash row
                    cls0 = sp.tile([cnt, K], u32, tag=f"c0{ti}")
                    nc.vector.tensor_scalar(
                        cls0[:, :],
                        d["aux"][:, 1:2].to_broadcast([cnt, K]), K, None,
                        op0=Alu.mult,
                    )
                    TT(out=posu[:, :], in0=posu[:, :], in1=cls0[:, :],
                       op=Alu.max)
                    b3 = sp.tile([cnt, 1], u32, tag=f"b3{ti}")
                    nc.gpsimd.iota(
                        b3[:, :], pattern=[[0, 1]],
                        base=off * 201, channel_multiplier=201,
                    )
                    TT(out=posu[:, :], in0=posu[:, :],
                       in1=b3[:, :].to_broadcast([cnt, K]), op=Alu.add)
                    d["posu"] = posu
                    row = sp.tile([cnt, K, 5], f32, tag=f"ro{ti}")
                    nc.vector.tensor_copy(row[:, :, 0], d["sorted"][:, :])
                    nc.vector.tensor_copy(row[:, :, 1:5], d["bx"][:, :, :])
                    d["row"] = row
                for sg in range(K):
                    for ti, (off, cnt) in enumerate(TILES):
                        d = D[ti]
                        nc.gpsimd.indirect_dma_start(
                            out=outp[:, :],
                            out_offset=IndirectOffsetOnAxis(
                                ap=d["posu"][:, sg:sg + 1], axis=0),
                            in_=d["row"][:, sg, :], in_offset=None,
                        )
    nc.compile()
    return nc


def _prep_core_inputs(loc_b, conf_b, loc2_b, conf2_b, dbox):
    """Pad per-core inputs to NPAD priors; build aux table."""
    pad = NPAD - N
    locp = np.pad(loc_b, ((0, 0), (0, pad), (0, 0)))
    loc2p = np.pad(loc2_b, ((0, 0), (0, pad), (0, 0)))
    cpad = np.zeros((conf_b.shape[0], pad, C), np.float32)
    cpad[:, :, 0] = 40.0
    cpad[:, :, 1:] = -40.0
    confp = np.concatenate([conf_b, cpad], axis=1)
    conf2p = np.concatenate([conf2_b, cpad], axis=1)
    dpad = np.zeros((pad, 4), np.float32)
    dpad[:, 2:] = 1e-3
    dboxp = np.concatenate([dbox, dpad], axis=0)
    aux = np.zeros((PAIRS, 2), np.uint32)
    for p in range(PAIRS):
        aux[p, 0] = (p // C) * W2
        aux[p, 1] = 1 if (p % C) == 0 else 0
    return {
        "loc1": np.ascontiguousarray(locp, np.float32),
        "loc2": np.ascontiguousarray(loc2p, np.float32),
        "conf1": np.ascontiguousarray(confp, np.float32),
        "conf2": np.ascontiguousarray(conf2p, np.float32),
        "dbox": np.ascontiguousarray(dboxp, np.float32),
        "aux": aux,
    }


def kernel(loc_data, conf_data, loc_data2, conf_data2, dbox_list):
    from concourse.bass_utils import run_bass_kernel_spmd

    loc_data = np.asarray(loc_data, np.float32)
    conf_data = np.asarray(conf_data, np.float32)
    loc_data2 = np.asarray(loc_data2, np.float32)
    conf_data2 = np.asarray(conf_data2, np.float32)
    dbox_list = np.asarray(dbox_list, np.float32)

    nc = build_nc()
    in_maps = []
    for k in range(8):
        sl = slice(k * IMG, (k + 1) * IMG)
        in_maps.append(
            _prep_core_inputs(
                loc_data[sl], conf_data[sl], loc_data2[sl], conf_data2[sl],
                dbox_list,
            )
        )
    res = run_bass_kernel_spmd(nc, in_maps, list(range(8))).results
    outs = []
    for k in range(8):
        o = np.asarray(res[k]["out"]).reshape(PAIRS, 201, 5)[:, :K, :]
        outs.append(o.reshape(IMG, C, K, 5))
    return np.concatenate(outs, axis=0)


# revision 16
# speedup vs baseline: 13.3794x; 1.0503x over previous
"""SSD detection post-processing (decode + softmax + per-class top-200 + NMS,
TTA-flip merge) as a Bass/Tile kernel for 8 Trainium2 NeuronCores.

Sharding: pure data parallel over the batch dim — core k handles images
8k..8k+7 (= 168 (image,class) pairs per core).

Per-core pipeline (all on device):
  1. decode both views' boxes (flip view 2), store to a DRAM box table
  2. softmax probs in prior-major layout; 32x32 stream-transpose + SBUF-SBUF
     DMA reassembly into pair-major score rows [pair, 17664]
  3. chunk-max (L=8) -> M [pair, 2208]; 25 rounds of max8/max_index/
     match_replace extract the 200 largest chunk maxes (provably a superset
     of the chunks holding the global top-200: if >200 chunks had max above
     the 201st value, there would be >200 elements above it)
  4. indirect-DMA gather of those chunks -> pool [pair, 1600]; 25 more
     extraction rounds give the exact sorted top-200 + pool slots
  5. map pool slots -> chunk ids -> score-column index; indirect-gather boxes
  6. upper-triangle IoU mask (iou > 0.45 as 1.45*inter - 0.45*(ai+aj) > 0);
     only Sm[i, k>i] is ever read by the scan, so the lower half is skipped
  7. greedy NMS via a running suppression vector: keep[i] = (supv[i]==0);
     supv[k>i] max= keep[i]*Sm[i, k>i] — two fused DVE ops per step
  8. compacted survivor rows scattered into the output via indirect DMA

The 168 pairs exceed the 128 SBUF partitions, so phase 2 runs as two
partition tiles (126 + 42 pairs). Their instruction streams are interleaved
op-by-op: the engines' in-order queues then always have an independent op
from the other tile to issue while a dependent chain waits, which hides
most dependency latency.
"""

import numpy as np

B = 64
N = 8732
C = 21
NPAD = 8832            # priors padded to 69*128
SLOTS = NPAD // 128    # 69
W2 = 2 * NPAD          # 17664 score columns per pair
IMG = 8                # images per core
PAIRS = IMG * C        # 168 pairs per core
CHUNK = 8
NCHUNK = W2 // CHUNK   # 2208
ROUNDS = 25            # 25*8 = 200
K = 200
RIOU = 8               # IoU block rows
TILES = ((0, 126), (126, 42))   # (pair offset, pair count) per partition tile


def build_nc(level=99):
    # level: phase-truncation for perf diagnosis (99 = full kernel).
    # 2=+M extraction; 3=+pool gather+sort; 4=+index mapping+box gather;
    # 5=+IoU mask; 6=+NMS scan; 7/99=+scatter.
    import concourse.bacc as bacc
    import concourse.bass as bass
    import concourse.mybir as mybir
    from concourse.bass import IndirectOffsetOnAxis
    from concourse.tile import TileContext

    f32 = mybir.dt.float32
    u32 = mybir.dt.uint32
    u8 = mybir.dt.uint8
    Alu = mybir.AluOpType
    Act = mybir.ActivationFunctionType
    Ax = mybir.AxisListType

    nc = bacc.Bacc()

    def TT(out, in0, in1, op):
        # TensorTensor's ISA struct can't encode multiple sync waits (codegen
        # "Too many sync wait commands"); TensorScalarPtr can, so emit every
        # tensor-tensor op as (in0 bypass 0.0) op in1.
        nc.vector.scalar_tensor_tensor(
            out=out, in0=in0, scalar=0.0, in1=in1, op0=Alu.bypass, op1=op,
        )

    loc1 = nc.declare_dram_parameter("loc1", [IMG, NPAD, 4], f32, isOutput=False)
    loc2 = nc.declare_dram_parameter("loc2", [IMG, NPAD, 4], f32, isOutput=False)
    conf1 = nc.declare_dram_parameter("conf1", [IMG, NPAD, C], f32, isOutput=False)
    conf2 = nc.declare_dram_parameter("conf2", [IMG, NPAD, C], f32, isOutput=False)
    dbox = nc.declare_dram_parameter("dbox", [NPAD, 4], f32, isOutput=False)
    # aux[pair] = (img*W2 base row into box table, 1 if class==0 else 0)
    aux = nc.declare_dram_parameter("aux", [PAIRS, 2], u32, isOutput=False)
    outp = nc.declare_dram_parameter("out", [PAIRS * 201, 5], f32, isOutput=True)

    scoresD = nc.dram_tensor("scoresD", [PAIRS * NCHUNK, CHUNK], f32)
    boxesD = nc.dram_tensor("boxesD", [IMG * W2, 4], f32)

    with TileContext(nc) as tc:
        with tc.tile_pool(name="Mpool", bufs=1) as mp:
            M_tiles = [
                mp.tile([cnt, NCHUNK], f32, tag=f"M{ti}", name=f"M{ti}")
                for ti, (off, cnt) in enumerate(TILES)
            ]
            # ---------------- phase 1: scores + boxes -----------------------
            with (
                tc.tile_pool(name="persist", bufs=1) as pp,
                tc.tile_pool(name="work", bufs=1) as wp,
            ):
                SA = pp.tile([126, W2], f32, tag="SA")
                SB = pp.tile([42, W2], f32, tag="SB")
                S_tiles = (SA, SB)

                dbox_t = pp.tile([128, SLOTS, 4], f32, tag="dbox")
                nc.sync.dma_start(
                    out=dbox_t[:, :, :],
                    in_=dbox.rearrange("(p s) c -> p s c", s=SLOTS),
                )

                for img in range(IMG):
                    for v, (locp, confp) in enumerate(
                        ((loc1, conf1), (loc2, conf2))
                    ):
                        # ---- decode ----
                        loc_t = wp.tile([128, SLOTS, 4], f32, tag="loc")
                        nc.sync.dma_start(
                            out=loc_t[:, :, :],
                            in_=locp[img].rearrange("(p s) c -> p s c", s=SLOTS),
                        )
                        box_t = wp.tile([128, SLOTS, 4], f32, tag="box")
                        wh_t = wp.tile([128, SLOTS, 2], f32, tag="wh")
                        t1_t = wp.tile([128, SLOTS, 2], f32, tag="dtmp")
                        # wh = dbox_wh * exp(0.2*loc_wh)
                        nc.scalar.activation(
                            wh_t[:, :, :], loc_t[:, :, 2:4], Act.Exp, scale=0.2
                        )
                        TT(
                            out=wh_t[:, :, :], in0=wh_t[:, :, :],
                            in1=dbox_t[:, :, 2:4], op=Alu.mult,
                        )
                        # cxy = dbox_xy + (loc_xy*0.1)*dbox_xy
                        nc.vector.tensor_scalar_mul(
                            t1_t[:, :, :], loc_t[:, :, :2], 0.1
                        )
                        TT(
                            out=t1_t[:, :, :], in0=t1_t[:, :, :],
                            in1=dbox_t[:, :, :2], op=Alu.mult,
                        )
                        TT(
                            out=t1_t[:, :, :], in0=t1_t[:, :, :],
                            in1=dbox_t[:, :, :2], op=Alu.add,
                        )
                        # mn = cxy - 0.5*wh ; mx = mn + wh
                        nc.vector.tensor_scalar_mul(
                            box_t[:, :, 2:4], wh_t[:, :, :], 0.5
                        )
                        TT(
                            out=box_t[:, :, 0:2], in0=t1_t[:, :, :],
                            in1=box_t[:, :, 2:4], op=Alu.subtract,
                        )
                        TT(
                            out=box_t[:, :, 2:4], in0=box_t[:, :, 0:2],
                            in1=wh_t[:, :, :], op=Alu.add,
                        )
                        if v == 1:
                            # flip: x1' = 1-x2, x2' = 1-x1
                            fx_t = wp.tile([128, SLOTS, 2], f32, tag="fx")
                            nc.vector.tensor_scalar(
                                fx_t[:, :, 0:1], box_t[:, :, 2:3], -1.0, 1.0,
                                op0=Alu.mult, op1=Alu.add,
                            )
                            nc.vector.tensor_scalar(
                                fx_t[:, :, 1:2], box_t[:, :, 0:1], -1.0, 1.0,
                                op0=Alu.mult, op1=Alu.add,
                            )
                            nc.vector.tensor_copy(
                                box_t[:, :, 0:1], fx_t[:, :, 0:1]
                            )
                            nc.vector.tensor_copy(
                                box_t[:, :, 2:3], fx_t[:, :, 1:2]
                            )
                        # boxesD row sigma = v*NPAD + pg*(SLOTS*32) + s*32 + l
                        for pg in range(4):
                            base = img * W2 + v * NPAD + pg * (SLOTS * 32)
                            nc.sync.dma_start(
                                out=boxesD[base:base + SLOTS * 32, :].rearrange(
                                    "(s l) c -> l s c", l=32
                                ),
                                in_=box_t[pg * 32:(pg + 1) * 32, :, :],
                            )

                        # ---- softmax (prior-major) ----
                        cf_t = wp.tile([128, SLOTS, C], f32, tag="cf")
                        nc.sync.dma_start(
                            out=cf_t[:, :, :],
                            in_=confp[img].rearrange("(p s) c -> p s c", s=SLOTS),
                        )
                        mx_t = wp.tile([128, SLOTS], f32, tag="mx")
                        nc.vector.tensor_reduce(
                            out=mx_t[:, :], in_=cf_t[:, :, :], axis=Ax.X,
                            op=Alu.max,
                        )
                        TT(
                            out=cf_t[:, :, :], in0=cf_t[:, :, :],
                            in1=mx_t[:, :, None].to_broadcast([128, SLOTS, C]),
                            op=Alu.subtract,
                        )
                        pr_t = wp.tile([128, SLOTS, 32], f32, tag="pr")
                        nc.vector.memset(pr_t[:, :, C:], 0.0)
                        nc.scalar.activation(
                            pr_t[:, :, :C], cf_t[:, :, :], Act.Exp
                        )
                        sm_t = wp.tile([128, SLOTS], f32, tag="sm")
                        nc.vector.tensor_reduce(
                            out=sm_t[:, :], in_=pr_t[:, :, :C], axis=Ax.X,
                            op=Alu.add,
                        )
                        nc.vector.reciprocal(sm_t[:, :], sm_t[:, :])
                        TT(
                            out=pr_t[:, :, :C], in0=pr_t[:, :, :C],
                            in1=sm_t[:, :, None].to_broadcast([128, SLOTS, C]),
                            op=Alu.mult,
                        )
                        # ---- 32x32 block transpose ----
                        tr_t = wp.tile([128, SLOTS, 32], f32, tag="tr")
                        nc.vector.transpose(
                            out=tr_t[:, :, :].rearrange("p s c -> p (s c)"),
                            in_=pr_t[:, :, :].rearrange("p s c -> p (s c)"),
                        )
                        # ---- SBUF->SBUF DMA into pair-major rows ----
                        if img < 6:
                            dst, row0 = SA, img * C
                        else:
                            dst, row0 = SB, (img - 6) * C
                        for pg in range(4):
                            nc.sync.dma_start(
                                out=dst[row0:row0 + C,
                                        v * NPAD + pg * (SLOTS * 32):
                                        v * NPAD + (pg + 1) * (SLOTS * 32)],
                                in_=tr_t[pg * 32: pg * 32 + C, :, :].rearrange(
                                    "c s l -> c (s l)"
                                ),
                            )

                # big copy of pair-major scores to DRAM + chunk max
                for (off, cnt), st, M_t in zip(TILES, S_tiles, M_tiles):
                    nc.sync.dma_start(
                        out=scoresD[off * NCHUNK:(off + cnt) * NCHUNK, :],
                        in_=st[:, :].rearrange("p (n k) -> p n k", k=CHUNK),
                    )
                    nc.vector.tensor_reduce(
                        out=M_t[:, :],
                        in_=st[:, :].rearrange("p (n k) -> p n k", k=CHUNK),
                        axis=Ax.X, op=Alu.max,
                    )
            # persist pool (SA/SB) freed here

            # zero the output (scatter only writes kept rows)
            with tc.tile_pool(name="zero", bufs=1) as zp:
                z_t = zp.tile([128, 201 * 5], f32, tag="z")
                nc.vector.memset(z_t[:, :], 0.0)
                for off, cnt in TILES:
                    nc.sync.dma_start(
                        out=outp[off * 201:(off + cnt) * 201, :].rearrange(
                            "(p r) c -> p (r c)", r=201),
                        in_=z_t[:cnt, :],
                    )

            # -------- phase 2: selection + NMS, tiles interleaved -----------
            if level < 2:
                nc.compile()
                return nc
            with tc.tile_pool(name="sel", bufs=1) as sp:
                D = []   # per-tile working tiles
                for ti, (off, cnt) in enumerate(TILES):
                    d = {}
                    d["cids"] = sp.tile([cnt, K], u32, tag=f"cid{ti}", name=f"cid{ti}")
                    d["v8"] = sp.tile([cnt, 8], f32, tag=f"v8{ti}", name=f"v8{ti}")
                    D.append(d)

                # ---- top-200 chunk extraction (interleaved rounds) ----
                for r in range(ROUNDS):
                    for ti, (off, cnt) in enumerate(TILES):
                        M_t, d = M_tiles[ti], D[ti]
                        nc.vector.max(out=d["v8"][:, :], in_=M_t[:, :])
                        nc.vector.max_index(
                            out=d["cids"][:, 8 * r:8 * r + 8],
                            in_max=d["v8"][:, :], in_values=M_t[:, :],
                        )
                        nc.vector.match_replace(
                            out=M_t[:, :], in_to_replace=d["v8"][:, :],
                            in_values=M_t[:, :], imm_value=-1.0,
                        )
                if level < 3:
                    nc.compile()
                    return nc

                # ---- chunk-id table to DRAM + pool gather ----
                for ti, (off, cnt) in enumerate(TILES):
                    d = D[ti]
                    d["base"] = sp.tile([cnt, 1], u32, tag=f"ba{ti}", name=f"ba{ti}")
                    nc.gpsimd.iota(
                        d["base"][:, :], pattern=[[0, 1]],
                        base=off * NCHUNK, channel_multiplier=NCHUNK,
                    )
                    d["gidx"] = sp.tile([cnt, K], u32, tag=f"gi{ti}", name=f"gi{ti}")
                    TT(
                        out=d["gidx"][:, :], in0=d["cids"][:, :],
                        in1=d["base"][:, :].to_broadcast([cnt, K]), op=Alu.add,
                    )
                    d["pool"] = sp.tile([cnt, K, CHUNK], f32, tag=f"po{ti}", name=f"po{ti}")
                for sg in range(K):
                    for ti, (off, cnt) in enumerate(TILES):
                        d = D[ti]
                        nc.gpsimd.indirect_dma_start(
                            out=d["pool"][:, sg, :], out_offset=None,
                            in_=scoresD[:, :],
                            in_offset=IndirectOffsetOnAxis(
                                ap=d["gidx"][:, sg:sg + 1], axis=0),
                        )

                # ---- exact sorted top-200 from the pool (interleaved) ----
                for ti, (off, cnt) in enumerate(TILES):
                    d = D[ti]
                    d["sorted"] = sp.tile([cnt, K], f32, tag=f"so{ti}", name=f"so{ti}")
                    d["ps"] = sp.tile([cnt, K], u32, tag=f"ps{ti}", name=f"ps{ti}")
                for r in range(ROUNDS):
                    for ti, (off, cnt) in enumerate(TILES):
                        d = D[ti]
                        poolf = d["pool"][:, :, :].rearrange("p n k -> p (n k)")
                        nc.vector.max(
                            out=d["sorted"][:, 8 * r:8 * r + 8], in_=poolf
                        )
                        nc.vector.max_index(
                            out=d["ps"][:, 8 * r:8 * r + 8],
                            in_max=d["sorted"][:, 8 * r:8 * r + 8],
                            in_values=poolf,
                        )
                        nc.vector.match_replace(
                            out=poolf,
                            in_to_replace=d["sorted"][:, 8 * r:8 * r + 8],
                            in_values=poolf, imm_value=-1.0,
                        )
                if level < 4:
                    nc.compile()
                    return nc

                # ---- pool slot -> chunk id via in-SBUF one-hot select ----
                # csel[p,k] = cids[p, sh[p,k]], domain 200: eq-mask against an
                # iota, mask*cids, reduce-max — on the vector engine (no DMA
                # round trip). Scratch: the IoU xa tile bitcast to u32.
                for ti, (off, cnt) in enumerate(TILES):
                    d = D[ti]
                    d["sh"] = sp.tile([cnt, K], u32, tag=f"sh{ti}", name=f"sh{ti}")
                    nc.vector.tensor_scalar(
                        d["sh"][:, :], d["ps"][:, :], 3, None,
                        op0=Alu.logical_shift_right,
                    )
                    d["iot"] = sp.tile([cnt, K], u32, tag=f"io{ti}", name=f"io{ti}")
                    nc.gpsimd.iota(
                        d["iot"][:, :], pattern=[[1, K]], base=0,
                        channel_multiplier=0,
                    )
                    d["csel"] = sp.tile([cnt, K], u32, tag=f"cs{ti}", name=f"cs{ti}")
                    d["xa"] = sp.tile([cnt, RIOU, K], f32, tag=f"xa{ti}", name=f"xa{ti}")
                    d["xb"] = sp.tile([cnt, RIOU, K], f32, tag=f"xb{ti}", name=f"xb{ti}")
                    d["xc"] = sp.tile([cnt, RIOU, K], f32, tag=f"xc{ti}", name=f"xc{ti}")
                for g0 in range(0, K, RIOU):
                    for ti, (off, cnt) in enumerate(TILES):
                        d = D[ti]
                        sh3 = [cnt, RIOU, K]
                        xu = d["xa"][:, :, :].bitcast(u32)
                        TT(
                            out=xu,
                            in0=d["sh"][:, g0:g0 + RIOU, None].to_broadcast(sh3),
                            in1=d["iot"][:, None, :].to_broadcast(sh3),
                            op=Alu.is_equal,
                        )
                        TT(
                            out=xu, in0=xu,
                            in1=d["cids"][:, None, :].to_broadcast(sh3),
                            op=Alu.mult,
                        )
                        nc.vector.tensor_reduce(
                            out=d["csel"][:, g0:g0 + RIOU], in_=xu, axis=Ax.X,
                            op=Alu.max,
                        )
                # sigma = cid*8 + (ps - (ps>>3)<<3); box row = sigma + img*W2
                for ti, (off, cnt) in enumerate(TILES):
                    d = D[ti]
                    d["lane"] = sp.tile([cnt, K], u32, tag=f"la{ti}", name=f"la{ti}")
                    nc.vector.tensor_scalar(
                        d["lane"][:, :], d["sh"][:, :], 3, None,
                        op0=Alu.logical_shift_left,
                    )
                    TT(
                        out=d["lane"][:, :], in0=d["ps"][:, :],
                        in1=d["lane"][:, :], op=Alu.subtract,
                    )
                    d["sig"] = sp.tile([cnt, K], u32, tag=f"sg{ti}", name=f"sg{ti}")
                    nc.vector.tensor_scalar(
                        d["sig"][:, :], d["csel"][:, :], 3, None,
                        op0=Alu.logical_shift_left,
                    )
                    TT(
                        out=d["sig"][:, :], in0=d["sig"][:, :],
                        in1=d["lane"][:, :], op=Alu.add,
                    )
                    d["aux"] = sp.tile([cnt, 2], u32, tag=f"ax{ti}", name=f"ax{ti}")
                    nc.sync.dma_start(
                        out=d["aux"][:, :], in_=aux[off:off + cnt, :]
                    )
                    TT(
                        out=d["sig"][:, :], in0=d["sig"][:, :],
                        in1=d["aux"][:, 0:1].to_broadcast([cnt, K]),
                        op=Alu.add,
                    )
                    d["bx"] = sp.tile([cnt, K, 4], f32, tag=f"bx{ti}", name=f"bx{ti}")
                    d["ar"] = sp.tile([cnt, K], f32, tag=f"ar{ti}", name=f"ar{ti}")
                    d["w0"] = sp.tile([cnt, K], f32, tag=f"w0{ti}", name=f"w0{ti}")
                # Gather boxes high-sg first and compute areas per 8-row
                # group as rows land; the IoU blocks run in reverse order so
                # block bi (rows/cols >= 8*bi) starts after the first few
                # groups instead of after the whole round.
                for g1 in range(K, 0, -RIOU):
                    g0 = g1 - RIOU
                    for sg in range(g1 - 1, g0 - 1, -1):
                        for ti, (off, cnt) in enumerate(TILES):
                            d = D[ti]
                            nc.gpsimd.indirect_dma_start(
                                out=d["bx"][:, sg, :], out_offset=None,
                                in_=boxesD[:, :],
                                in_offset=IndirectOffsetOnAxis(
                                    ap=d["sig"][:, sg:sg + 1], axis=0),
                            )
                    for ti, (off, cnt) in enumerate(TILES):
                        d = D[ti]
                        bxg = d["bx"][:, g0:g1, :]
                        TT(
                            out=d["w0"][:, g0:g1], in0=bxg[:, :, 2],
                            in1=bxg[:, :, 0], op=Alu.subtract,
                        )
                        TT(
                            out=d["ar"][:, g0:g1], in0=bxg[:, :, 3],
                            in1=bxg[:, :, 1], op=Alu.subtract,
                        )
                        TT(
                            out=d["ar"][:, g0:g1], in0=d["ar"][:, g0:g1],
                            in1=d["w0"][:, g0:g1], op=Alu.mult,
                        )
                if level < 5:
                    nc.compile()
                    return nc

                # ---- upper-triangle IoU mask (interleaved blocks) ----
                # S[i,k] = 1.45*inter - 0.45*(ai+ak) > 0, computed only for
                # k >= r0 of each row block [r0, r0+RIOU) — the NMS scan
                # reads only k > i.
                for ti, (off, cnt) in enumerate(TILES):
                    d = D[ti]
                    d["Sm"] = sp.tile([cnt, K, K], u8, tag=f"Sm{ti}", name=f"Sm{ti}")
                for bi in range(K // RIOU - 1, -1, -1):
                    r0 = RIOU * bi
                    W = K - r0
                    for ti, (off, cnt) in enumerate(TILES):
                        d = D[ti]
                        bx = d["bx"]
                        rows = bx[:, r0:r0 + RIOU, :]
                        sh3 = [cnt, RIOU, W]
                        xa = d["xa"][:, :, :W]
                        xb = d["xb"][:, :, :W]
                        xc = d["xc"][:, :, :W]
                        TT(
                            out=xc,
                            in0=rows[:, :, 1:2].to_broadcast(sh3),
                            in1=bx[:, None, r0:, 1].to_broadcast(sh3),
                            op=Alu.max,
                        )
                        TT(
                            out=xb,
                            in0=rows[:, :, 3:4].to_broadcast(sh3),
                            in1=bx[:, None, r0:, 3].to_broadcast(sh3),
                            op=Alu.min,
                        )
                        TT(out=xb, in0=xb, in1=xc, op=Alu.subtract)
                        nc.vector.tensor_scalar_max(xb, xb, 0.0)
                        TT(
                            out=xa,
                            in0=rows[:, :, 0:1].to_broadcast(sh3),
                            in1=bx[:, None, r0:, 0].to_broadcast(sh3),
                            op=Alu.max,
                        )
                        TT(
                            out=xc,
                            in0=rows[:, :, 2:3].to_broadcast(sh3),
                            in1=bx[:, None, r0:, 2].to_broadcast(sh3),
                            op=Alu.min,
                        )
                        TT(out=xa, in0=xc, in1=xa, op=Alu.subtract)
                        nc.vector.tensor_scalar_max(xa, xa, 0.0)
                        TT(out=xa, in0=xa, in1=xb, op=Alu.mult)
                        TT(
                            out=xb,
                            in0=d["ar"][:, r0:r0 + RIOU, None].to_broadcast(
                                sh3),
                            in1=d["ar"][:, None, r0:].to_broadcast(sh3),
                            op=Alu.add,
                        )
                        nc.vector.tensor_scalar_mul(xa, xa, 1.45)
                        nc.vector.scalar_tensor_tensor(
                            out=xa, in0=xb, scalar=-0.45, in1=xa,
                            op0=Alu.mult, op1=Alu.add,
                        )
                        nc.vector.tensor_scalar(
                            d["Sm"][:, r0:r0 + RIOU, r0:], xa, 0.0, None,
                            op0=Alu.is_gt,
                        )
                if level < 6:
                    nc.compile()
                    return nc

                # ---- greedy NMS scan (running suppression vector) ----
                for ti, (off, cnt) in enumerate(TILES):
                    d = D[ti]
                    d["keep"] = sp.tile([cnt, K], u8, tag=f"ke{ti}", name=f"ke{ti}")
                    d["supv"] = sp.tile([cnt, K], u8, tag=f"sv{ti}", name=f"sv{ti}")
                    nc.vector.memset(d["supv"][:, :], 0)
                for i in range(K):
                    for ti, (off, cnt) in enumerate(TILES):
                        d = D[ti]
                        nc.vector.tensor_scalar(
                            d["keep"][:, i:i + 1], d["supv"][:, i:i + 1],
                            0, None, op0=Alu.is_equal,
                        )
                        if i < K - 1:
                            nc.vector.scalar_tensor_tensor(
                                out=d["supv"][:, i + 1:],
                                in0=d["Sm"][:, i, i + 1:],
                                scalar=d["keep"][:, i:i + 1],
                                in1=d["supv"][:, i + 1:],
                                op0=Alu.mult, op1=Alu.max,
                            )
                if level < 7:
                    nc.compile()
                    return nc

                # ---- output scatter ----
                for ti, (off, cnt) in enumerate(TILES):
                    d = D[ti]
                    keepf = sp.tile([cnt, K], f32, tag=f"kf{ti}")
                    nc.vector.tensor_copy(keepf[:, :], d["keep"][:, :])
                    pos = sp.tile([cnt, K], f32, tag=f"pf{ti}")
                    nc.vector.tensor_tensor_scan(
                        out=pos[:, :], data0=keepf[:, :],
                        data1=keepf[:, :], initial=-1.0,
                        op0=Alu.add, op1=Alu.bypass,
                    )
                    posx = sp.tile([cnt, K], f32, tag=f"px{ti}")
                    nc.vector.memset(posx[:, :], float(K))
                    nc.vector.copy_predicated(
                        posx[:, :], d["keep"][:, :], pos[:, :]
                    )
                    posu = sp.tile([cnt, K], u32, tag=f"pu{ti}")
                    nc.vector.tensor_copy(posu[:, :], posx[:, :])
                    # class-0 pairs always go to the trash row
                    cls0 = sp.tile([cnt, K], u32, tag=f"c0{ti}")
                    nc.vector.tensor_scalar(
                        cls0[:, :],
                        d["aux"][:, 1:2].to_broadcast([cnt, K]), K, None,
                        op0=Alu.mult,
                    )
                    TT(out=posu[:, :], in0=posu[:, :], in1=cls0[:, :],
                       op=Alu.max)
                    b3 = sp.tile([cnt, 1], u32, tag=f"b3{ti}")
                    nc.gpsimd.iota(
                        b3[:, :], pattern=[[0, 1]],
                        base=off * 201, channel_multiplier=201,
                    )
                    TT(out=posu[:, :], in0=posu[:, :],
                       in1=b3[:, :].to_broadcast([cnt, K]), op=Alu.add)
                    d["posu"] = posu
                    row = sp.tile([cnt, K, 5], f32, tag=f"ro{ti}")
                    nc.vector.tensor_copy(row[:, :, 0], d["sorted"][:, :])
                    nc.vector.tensor_copy(row[:, :, 1:5], d["bx"][:, :, :])
                    d["row"] = row
                for sg in range(K):
                    for ti, (off, cnt) in enumerate(TILES):
                        d = D[ti]
                        nc.gpsimd.indirect_dma_start(
                            out=outp[:, :],
                            out_offset=IndirectOffsetOnAxis(
                                ap=d["posu"][:, sg:sg + 1], axis=0),
                            in_=d["row"][:, sg, :], in_offset=None,
                        )
    nc.compile()
    return nc


def _prep_core_inputs(loc_b, conf_b, loc2_b, conf2_b, dbox):
    """Pad per-core inputs to NPAD priors; build aux table."""
    pad = NPAD - N
    locp = np.pad(loc_b, ((0, 0), (0, pad), (0, 0)))
    loc2p = np.pad(loc2_b, ((0, 0), (0, pad), (0, 0)))
    cpad = np.zeros((conf_b.shape[0], pad, C), np.float32)
    cpad[:, :, 0] = 40.0
    cpad[:, :, 1:] = -40.0
    confp = np.concatenate([conf_b, cpad], axis=1)
    conf2p = np.concatenate([conf2_b, cpad], axis=1)
    dpad = np.zeros((pad, 4), np.float32)
    dpad[:, 2:] = 1e-3
    dboxp = np.concatenate([dbox, dpad], axis=0)
    aux = np.zeros((PAIRS, 2), np.uint32)
    for p in range(PAIRS):
        aux[p, 0] = (p // C) * W2
        aux[p, 1] = 1 if (p % C) == 0 else 0
    return {
        "loc1": np.ascontiguousarray(locp, np.float32),
        "loc2": np.ascontiguousarray(loc2p, np.float32),
        "conf1": np.ascontiguousarray(confp, np.float32),
        "conf2": np.ascontiguousarray(conf2p, np.float32),
        "dbox": np.ascontiguousarray(dboxp, np.float32),
        "aux": aux,
    }


def kernel(loc_data, conf_data, loc_data2, conf_data2, dbox_list):
    from concourse.bass_utils import run_bass_kernel_spmd

    loc_data = np.asarray(loc_data, np.float32)
    conf_data = np.asarray(conf_data, np.float32)
    loc_data2 = np.asarray(loc_data2, np.float32)
    conf_data2 = np.asarray(conf_data2, np.float32)
    dbox_list = np.asarray(dbox_list, np.float32)

    nc = build_nc()
    in_maps = []
    for k in range(8):
        sl = slice(k * IMG, (k + 1) * IMG)
        in_maps.append(
            _prep_core_inputs(
                loc_data[sl], conf_data[sl], loc_data2[sl], conf_data2[sl],
                dbox_list,
            )
        )
    res = run_bass_kernel_spmd(nc, in_maps, list(range(8))).results
    outs = []
    for k in range(8):
        o = np.asarray(res[k]["out"]).reshape(PAIRS, 201, 5)[:, :K, :]
        outs.append(o.reshape(IMG, C, K, 5))
    return np.concatenate(outs, axis=0)


# revision 17
# speedup vs baseline: 13.6254x; 1.0184x over previous
"""SSD detection post-processing (decode + softmax + per-class top-200 + NMS,
TTA-flip merge) as a Bass/Tile kernel for 8 Trainium2 NeuronCores.

Sharding: pure data parallel over the batch dim — core k handles images
8k..8k+7 (= 168 (image,class) pairs per core).

Per-core pipeline (all on device):
  1. decode both views' boxes (flip view 2), store to a DRAM box table
  2. softmax probs in prior-major layout; 32x32 stream-transpose + SBUF-SBUF
     DMA reassembly into pair-major score rows [pair, 17664]
  3. chunk-max (L=8) -> M [pair, 2208]; 25 rounds of max8/max_index/
     match_replace extract the 200 largest chunk maxes (provably a superset
     of the chunks holding the global top-200: if >200 chunks had max above
     the 201st value, there would be >200 elements above it)
  4. indirect-DMA gather of those chunks -> pool [pair, 1600]; 25 more
     extraction rounds give the exact sorted top-200 + pool slots
  5. map pool slots -> chunk ids -> score-column index; indirect-gather boxes
  6. upper-triangle IoU mask (iou > 0.45 as 1.45*inter - 0.45*(ai+aj) > 0);
     only Sm[i, k>i] is ever read by the scan, so the lower half is skipped
  7. greedy NMS via a running suppression vector: keep[i] = (supv[i]==0);
     supv[k>i] max= keep[i]*Sm[i, k>i] — two fused DVE ops per step
  8. compacted survivor rows scattered into the output via indirect DMA

The 168 pairs exceed the 128 SBUF partitions, so phase 2 runs as two
partition tiles (126 + 42 pairs). Their instruction streams are interleaved
op-by-op: the engines' in-order queues then always have an independent op
from the other tile to issue while a dependent chain waits, which hides
most dependency latency.
"""

import numpy as np

B = 64
N = 8732
C = 21
NPAD = 8832            # priors padded to 69*128
SLOTS = NPAD // 128    # 69
W2 = 2 * NPAD          # 17664 score columns per pair
IMG = 8                # images per core
PAIRS = IMG * C        # 168 pairs per core
CHUNK = 8
NCHUNK = W2 // CHUNK   # 2208
ROUNDS = 25            # 25*8 = 200
K = 200
RIOU = 8               # IoU block rows
TILES = ((0, 126), (126, 42))   # (pair offset, pair count) per partition tile


def build_nc(level=99):
    # level: phase-truncation for perf diagnosis (99 = full kernel).
    # 2=+M extraction; 3=+pool gather+sort; 4=+index mapping+box gather;
    # 5=+IoU mask; 6=+NMS scan; 7/99=+scatter.
    import concourse.bacc as bacc
    import concourse.bass as bass
    import concourse.mybir as mybir
    from concourse.bass import IndirectOffsetOnAxis
    from concourse.tile import TileContext

    f32 = mybir.dt.float32
    u32 = mybir.dt.uint32
    u8 = mybir.dt.uint8
    Alu = mybir.AluOpType
    Act = mybir.ActivationFunctionType
    Ax = mybir.AxisListType

    nc = bacc.Bacc()

    def TT(out, in0, in1, op):
        # TensorTensor's ISA struct can't encode multiple sync waits (codegen
        # "Too many sync wait commands"); TensorScalarPtr can, so emit every
        # tensor-tensor op as (in0 bypass 0.0) op in1.
        nc.vector.scalar_tensor_tensor(
            out=out, in0=in0, scalar=0.0, in1=in1, op0=Alu.bypass, op1=op,
        )

    loc1 = nc.declare_dram_parameter("loc1", [IMG, NPAD, 4], f32, isOutput=False)
    loc2 = nc.declare_dram_parameter("loc2", [IMG, NPAD, 4], f32, isOutput=False)
    conf1 = nc.declare_dram_parameter("conf1", [IMG, NPAD, C], f32, isOutput=False)
    conf2 = nc.declare_dram_parameter("conf2", [IMG, NPAD, C], f32, isOutput=False)
    dbox = nc.declare_dram_parameter("dbox", [NPAD, 4], f32, isOutput=False)
    # aux[pair] = (img*W2 base row into box table, 1 if class==0 else 0)
    aux = nc.declare_dram_parameter("aux", [PAIRS, 2], u32, isOutput=False)
    outp = nc.declare_dram_parameter("out", [PAIRS * 201, 5], f32, isOutput=True)

    scoresD = nc.dram_tensor("scoresD", [PAIRS * NCHUNK, CHUNK], f32)
    boxesD = nc.dram_tensor("boxesD", [IMG * W2, 4], f32)

    with TileContext(nc) as tc:
        with tc.tile_pool(name="Mpool", bufs=1) as mp:
            M_tiles = [
                mp.tile([cnt, NCHUNK], f32, tag=f"M{ti}", name=f"M{ti}")
                for ti, (off, cnt) in enumerate(TILES)
            ]
            # ---------------- phase 1: scores + boxes -----------------------
            with (
                tc.tile_pool(name="persist", bufs=1) as pp,
                tc.tile_pool(name="work", bufs=1) as wp,
            ):
                SA = pp.tile([126, W2], f32, tag="SA")
                SB = pp.tile([42, W2], f32, tag="SB")
                S_tiles = (SA, SB)

                dbox_t = pp.tile([128, SLOTS, 4], f32, tag="dbox")
                nc.sync.dma_start(
                    out=dbox_t[:, :, :],
                    in_=dbox.rearrange("(p s) c -> p s c", s=SLOTS),
                )

                for img in range(IMG):
                    for v, (locp, confp) in enumerate(
                        ((loc1, conf1), (loc2, conf2))
                    ):
                        # ---- decode ----
                        loc_t = wp.tile([128, SLOTS, 4], f32, tag="loc")
                        nc.sync.dma_start(
                            out=loc_t[:, :, :],
                            in_=locp[img].rearrange("(p s) c -> p s c", s=SLOTS),
                        )
                        box_t = wp.tile([128, SLOTS, 4], f32, tag="box")
                        wh_t = wp.tile([128, SLOTS, 2], f32, tag="wh")
                        t1_t = wp.tile([128, SLOTS, 2], f32, tag="dtmp")
                        # wh = dbox_wh * exp(0.2*loc_wh)
                        nc.scalar.activation(
                            wh_t[:, :, :], loc_t[:, :, 2:4], Act.Exp, scale=0.2
                        )
                        TT(
                            out=wh_t[:, :, :], in0=wh_t[:, :, :],
                            in1=dbox_t[:, :, 2:4], op=Alu.mult,
                        )
                        # cxy = dbox_xy + (loc_xy*0.1)*dbox_xy
                        nc.vector.tensor_scalar_mul(
                            t1_t[:, :, :], loc_t[:, :, :2], 0.1
                        )
                        TT(
                            out=t1_t[:, :, :], in0=t1_t[:, :, :],
                            in1=dbox_t[:, :, :2], op=Alu.mult,
                        )
                        TT(
                            out=t1_t[:, :, :], in0=t1_t[:, :, :],
                            in1=dbox_t[:, :, :2], op=Alu.add,
                        )
                        # mn = cxy - 0.5*wh ; mx = mn + wh
                        nc.vector.tensor_scalar_mul(
                            box_t[:, :, 2:4], wh_t[:, :, :], 0.5
                        )
                        TT(
                            out=box_t[:, :, 0:2], in0=t1_t[:, :, :],
                            in1=box_t[:, :, 2:4], op=Alu.subtract,
                        )
                        TT(
                            out=box_t[:, :, 2:4], in0=box_t[:, :, 0:2],
                            in1=wh_t[:, :, :], op=Alu.add,
                        )
                        if v == 1:
                            # flip: x1' = 1-x2, x2' = 1-x1
                            fx_t = wp.tile([128, SLOTS, 2], f32, tag="fx")
                            nc.vector.tensor_scalar(
                                fx_t[:, :, 0:1], box_t[:, :, 2:3], -1.0, 1.0,
                                op0=Alu.mult, op1=Alu.add,
                            )
                            nc.vector.tensor_scalar(
                                fx_t[:, :, 1:2], box_t[:, :, 0:1], -1.0, 1.0,
                                op0=Alu.mult, op1=Alu.add,
                            )
                            nc.vector.tensor_copy(
                                box_t[:, :, 0:1], fx_t[:, :, 0:1]
                            )
                            nc.vector.tensor_copy(
                                box_t[:, :, 2:3], fx_t[:, :, 1:2]
                            )
                        # boxesD row sigma = v*NPAD + pg*(SLOTS*32) + s*32 + l
                        for pg in range(4):
                            base = img * W2 + v * NPAD + pg * (SLOTS * 32)
                            nc.sync.dma_start(
                                out=boxesD[base:base + SLOTS * 32, :].rearrange(
                                    "(s l) c -> l s c", l=32
                                ),
                                in_=box_t[pg * 32:(pg + 1) * 32, :, :],
                            )

                        # ---- softmax (prior-major) ----
                        cf_t = wp.tile([128, SLOTS, C], f32, tag="cf")
                        nc.sync.dma_start(
                            out=cf_t[:, :, :],
                            in_=confp[img].rearrange("(p s) c -> p s c", s=SLOTS),
                        )
                        mx_t = wp.tile([128, SLOTS], f32, tag="mx")
                        nc.vector.tensor_reduce(
                            out=mx_t[:, :], in_=cf_t[:, :, :], axis=Ax.X,
                            op=Alu.max,
                        )
                        TT(
                            out=cf_t[:, :, :], in0=cf_t[:, :, :],
                            in1=mx_t[:, :, None].to_broadcast([128, SLOTS, C]),
                            op=Alu.subtract,
                        )
                        pr_t = wp.tile([128, SLOTS, 32], f32, tag="pr")
                        nc.vector.memset(pr_t[:, :, C:], 0.0)
                        nc.scalar.activation(
                            pr_t[:, :, :C], cf_t[:, :, :], Act.Exp
                        )
                        sm_t = wp.tile([128, SLOTS], f32, tag="sm")
                        nc.vector.tensor_reduce(
                            out=sm_t[:, :], in_=pr_t[:, :, :C], axis=Ax.X,
                            op=Alu.add,
                        )
                        nc.vector.reciprocal(sm_t[:, :], sm_t[:, :])
                        TT(
                            out=pr_t[:, :, :C], in0=pr_t[:, :, :C],
                            in1=sm_t[:, :, None].to_broadcast([128, SLOTS, C]),
                            op=Alu.mult,
                        )
                        # ---- 32x32 block transpose ----
                        tr_t = wp.tile([128, SLOTS, 32], f32, tag="tr")
                        nc.vector.transpose(
                            out=tr_t[:, :, :].rearrange("p s c -> p (s c)"),
                            in_=pr_t[:, :, :].rearrange("p s c -> p (s c)"),
                        )
                        # ---- SBUF->SBUF DMA into pair-major rows ----
                        if img < 6:
                            dst, row0 = SA, img * C
                        else:
                            dst, row0 = SB, (img - 6) * C
                        for pg in range(4):
                            nc.sync.dma_start(
                                out=dst[row0:row0 + C,
                                        v * NPAD + pg * (SLOTS * 32):
                                        v * NPAD + (pg + 1) * (SLOTS * 32)],
                                in_=tr_t[pg * 32: pg * 32 + C, :, :].rearrange(
                                    "c s l -> c (s l)"
                                ),
                            )

                # big copy of pair-major scores to DRAM + chunk max
                for (off, cnt), st, M_t in zip(TILES, S_tiles, M_tiles):
                    nc.sync.dma_start(
                        out=scoresD[off * NCHUNK:(off + cnt) * NCHUNK, :],
                        in_=st[:, :].rearrange("p (n k) -> p n k", k=CHUNK),
                    )
                    nc.vector.tensor_reduce(
                        out=M_t[:, :],
                        in_=st[:, :].rearrange("p (n k) -> p n k", k=CHUNK),
                        axis=Ax.X, op=Alu.max,
                    )
            # persist pool (SA/SB) freed here

            # zero the output (scatter only writes kept rows)
            with tc.tile_pool(name="zero", bufs=1) as zp:
                z_t = zp.tile([128, 201 * 5], f32, tag="z")
                nc.vector.memset(z_t[:, :], 0.0)
                for off, cnt in TILES:
                    nc.sync.dma_start(
                        out=outp[off * 201:(off + cnt) * 201, :].rearrange(
                            "(p r) c -> p (r c)", r=201),
                        in_=z_t[:cnt, :],
                    )

            # -------- phase 2: selection + NMS, tiles interleaved -----------
            if level < 2:
                nc.compile()
                return nc
            with tc.tile_pool(name="sel", bufs=1) as sp:
                D = []   # per-tile working tiles
                for ti, (off, cnt) in enumerate(TILES):
                    d = {}
                    d["cids"] = sp.tile([cnt, K], u32, tag=f"cid{ti}", name=f"cid{ti}")
                    d["v8"] = sp.tile([cnt, 8], f32, tag=f"v8{ti}", name=f"v8{ti}")
                    D.append(d)

                # ---- top-200 chunk extraction (interleaved rounds) ----
                for r in range(ROUNDS):
                    for ti, (off, cnt) in enumerate(TILES):
                        M_t, d = M_tiles[ti], D[ti]
                        nc.vector.max(out=d["v8"][:, :], in_=M_t[:, :])
                        nc.vector.max_index(
                            out=d["cids"][:, 8 * r:8 * r + 8],
                            in_max=d["v8"][:, :], in_values=M_t[:, :],
                        )
                        nc.vector.match_replace(
                            out=M_t[:, :], in_to_replace=d["v8"][:, :],
                            in_values=M_t[:, :], imm_value=-1.0,
                        )
                if level < 3:
                    nc.compile()
                    return nc

                # ---- chunk-id table to DRAM + pool gather ----
                for ti, (off, cnt) in enumerate(TILES):
                    d = D[ti]
                    d["base"] = sp.tile([cnt, 1], u32, tag=f"ba{ti}", name=f"ba{ti}")
                    nc.gpsimd.iota(
                        d["base"][:, :], pattern=[[0, 1]],
                        base=off * NCHUNK, channel_multiplier=NCHUNK,
                    )
                    d["gidx"] = sp.tile([cnt, K], u32, tag=f"gi{ti}", name=f"gi{ti}")
                    TT(
                        out=d["gidx"][:, :], in0=d["cids"][:, :],
                        in1=d["base"][:, :].to_broadcast([cnt, K]), op=Alu.add,
                    )
                    d["pool"] = sp.tile([cnt, K, CHUNK], f32, tag=f"po{ti}", name=f"po{ti}")
                for sg in range(K):
                    for ti, (off, cnt) in enumerate(TILES):
                        d = D[ti]
                        nc.gpsimd.indirect_dma_start(
                            out=d["pool"][:, sg, :], out_offset=None,
                            in_=scoresD[:, :],
                            in_offset=IndirectOffsetOnAxis(
                                ap=d["gidx"][:, sg:sg + 1], axis=0),
                        )

                # ---- exact sorted top-200 from the pool (interleaved) ----
                for ti, (off, cnt) in enumerate(TILES):
                    d = D[ti]
                    d["sorted"] = sp.tile([cnt, K], f32, tag=f"so{ti}", name=f"so{ti}")
                    d["ps"] = sp.tile([cnt, K], u32, tag=f"ps{ti}", name=f"ps{ti}")
                for r in range(ROUNDS):
                    for ti, (off, cnt) in enumerate(TILES):
                        d = D[ti]
                        poolf = d["pool"][:, :, :].rearrange("p n k -> p (n k)")
                        nc.vector.max(
                            out=d["sorted"][:, 8 * r:8 * r + 8], in_=poolf
                        )
                        nc.vector.max_index(
                            out=d["ps"][:, 8 * r:8 * r + 8],
                            in_max=d["sorted"][:, 8 * r:8 * r + 8],
                            in_values=poolf,
                        )
                        nc.vector.match_replace(
                            out=poolf,
                            in_to_replace=d["sorted"][:, 8 * r:8 * r + 8],
                            in_values=poolf, imm_value=-1.0,
                        )
                if level < 4:
                    nc.compile()
                    return nc

                # ---- pool slot -> chunk id via in-SBUF one-hot select ----
                # csel[p,k] = cids[p, sh[p,k]], domain 200: eq-mask against an
                # iota, mask*cids, reduce-max — on the vector engine (no DMA
                # round trip). Scratch: the IoU xa tile bitcast to u32.
                for ti, (off, cnt) in enumerate(TILES):
                    d = D[ti]
                    d["sh"] = sp.tile([cnt, K], u32, tag=f"sh{ti}", name=f"sh{ti}")
                    nc.vector.tensor_scalar(
                        d["sh"][:, :], d["ps"][:, :], 3, None,
                        op0=Alu.logical_shift_right,
                    )
                    d["iot"] = sp.tile([cnt, K], u32, tag=f"io{ti}", name=f"io{ti}")
                    nc.gpsimd.iota(
                        d["iot"][:, :], pattern=[[1, K]], base=0,
                        channel_multiplier=0,
                    )
                    d["csel"] = sp.tile([cnt, K], u32, tag=f"cs{ti}", name=f"cs{ti}")
                    d["xa"] = sp.tile([cnt, RIOU, K], f32, tag=f"xa{ti}", name=f"xa{ti}")
                    d["xb"] = sp.tile([cnt, RIOU, K], f32, tag=f"xb{ti}", name=f"xb{ti}")
                    d["xc"] = sp.tile([cnt, RIOU, K], f32, tag=f"xc{ti}", name=f"xc{ti}")
                for g0 in range(0, K, RIOU):
                    for ti, (off, cnt) in enumerate(TILES):
                        d = D[ti]
                        sh3 = [cnt, RIOU, K]
                        xu = d["xa"][:, :, :].bitcast(u32)
                        TT(
                            out=xu,
                            in0=d["sh"][:, g0:g0 + RIOU, None].to_broadcast(sh3),
                            in1=d["iot"][:, None, :].to_broadcast(sh3),
                            op=Alu.is_equal,
                        )
                        TT(
                            out=xu, in0=xu,
                            in1=d["cids"][:, None, :].to_broadcast(sh3),
                            op=Alu.mult,
                        )
                        nc.vector.tensor_reduce(
                            out=d["csel"][:, g0:g0 + RIOU], in_=xu, axis=Ax.X,
                            op=Alu.max,
                        )
                # sigma = cid*8 + (ps - (ps>>3)<<3); box row = sigma + img*W2
                for ti, (off, cnt) in enumerate(TILES):
                    d = D[ti]
                    d["lane"] = sp.tile([cnt, K], u32, tag=f"la{ti}", name=f"la{ti}")
                    nc.vector.tensor_scalar(
                        d["lane"][:, :], d["sh"][:, :], 3, None,
                        op0=Alu.logical_shift_left,
                    )
                    TT(
                        out=d["lane"][:, :], in0=d["ps"][:, :],
                        in1=d["lane"][:, :], op=Alu.subtract,
                    )
                    d["sig"] = sp.tile([cnt, K], u32, tag=f"sg{ti}", name=f"sg{ti}")
                    nc.vector.tensor_scalar(
                        d["sig"][:, :], d["csel"][:, :], 3, None,
                        op0=Alu.logical_shift_left,
                    )
                    TT(
                        out=d["sig"][:, :], in0=d["sig"][:, :],
                        in1=d["lane"][:, :], op=Alu.add,
                    )
                    d["aux"] = sp.tile([cnt, 2], u32, tag=f"ax{ti}", name=f"ax{ti}")
                    nc.sync.dma_start(
                        out=d["aux"][:, :], in_=aux[off:off + cnt, :]
                    )
                    TT(
                        out=d["sig"][:, :], in0=d["sig"][:, :],
                        in1=d["aux"][:, 0:1].to_broadcast([cnt, K]),
                        op=Alu.add,
                    )
                    d["bx"] = sp.tile([cnt, K, 4], f32, tag=f"bx{ti}", name=f"bx{ti}")
                    d["ar"] = sp.tile([cnt, K], f32, tag=f"ar{ti}", name=f"ar{ti}")
                    d["w0"] = sp.tile([cnt, K], f32, tag=f"w0{ti}", name=f"w0{ti}")
                # Gather boxes high-sg first and compute areas per 8-row
                # group as rows land; the IoU blocks run in reverse order so
                # block bi (rows/cols >= 8*bi) starts after the first few
                # groups instead of after the whole round.
                for g1 in range(K, 0, -RIOU):
                    g0 = g1 - RIOU
                    for sg in range(g1 - 1, g0 - 1, -1):
                        for ti, (off, cnt) in enumerate(TILES):
                            d = D[ti]
                            nc.gpsimd.indirect_dma_start(
                                out=d["bx"][:, sg, :], out_offset=None,
                                in_=boxesD[:, :],
                                in_offset=IndirectOffsetOnAxis(
                                    ap=d["sig"][:, sg:sg + 1], axis=0),
                            )
                    for ti, (off, cnt) in enumerate(TILES):
                        d = D[ti]
                        bxg = d["bx"][:, g0:g1, :]
                        TT(
                            out=d["w0"][:, g0:g1], in0=bxg[:, :, 2],
                            in1=bxg[:, :, 0], op=Alu.subtract,
                        )
                        TT(
                            out=d["ar"][:, g0:g1], in0=bxg[:, :, 3],
                            in1=bxg[:, :, 1], op=Alu.subtract,
                        )
                        TT(
                            out=d["ar"][:, g0:g1], in0=d["ar"][:, g0:g1],
                            in1=d["w0"][:, g0:g1], op=Alu.mult,
                        )
                if level < 5:
                    nc.compile()
                    return nc

                # ---- upper-triangle IoU mask (interleaved blocks) ----
                # S[i,k] = 1.45*inter - 0.45*(ai+ak) > 0, computed only for
                # k >= r0 of each row block [r0, r0+RIOU) — the NMS scan
                # reads only k > i.
                for ti, (off, cnt) in enumerate(TILES):
                    d = D[ti]
                    d["Sm"] = sp.tile([cnt, K, K], u8, tag=f"Sm{ti}", name=f"Sm{ti}")
                    row = sp.tile([cnt, K, 5], f32, tag=f"ro{ti}", name=f"ro{ti}")
                    nc.vector.tensor_copy(row[:, :, 0], d["sorted"][:, :])
                    nc.vector.tensor_copy(row[:, :, 1:5], d["bx"][:, :, :])
                    d["row"] = row
                for bi in range(K // RIOU - 1, -1, -1):
                    r0 = RIOU * bi
                    W = K - r0
                    for ti, (off, cnt) in enumerate(TILES):
                        d = D[ti]
                        bx = d["bx"]
                        rows = bx[:, r0:r0 + RIOU, :]
                        sh3 = [cnt, RIOU, W]
                        xa = d["xa"][:, :, :W]
                        xb = d["xb"][:, :, :W]
                        xc = d["xc"][:, :, :W]
                        TT(
                            out=xc,
                            in0=rows[:, :, 1:2].to_broadcast(sh3),
                            in1=bx[:, None, r0:, 1].to_broadcast(sh3),
                            op=Alu.max,
                        )
                        TT(
                            out=xb,
                            in0=rows[:, :, 3:4].to_broadcast(sh3),
                            in1=bx[:, None, r0:, 3].to_broadcast(sh3),
                            op=Alu.min,
                        )
                        TT(out=xb, in0=xb, in1=xc, op=Alu.subtract)
                        nc.vector.tensor_scalar_max(xb, xb, 0.0)
                        TT(
                            out=xa,
                            in0=rows[:, :, 0:1].to_broadcast(sh3),
                            in1=bx[:, None, r0:, 0].to_broadcast(sh3),
                            op=Alu.max,
                        )
                        TT(
                            out=xc,
                            in0=rows[:, :, 2:3].to_broadcast(sh3),
                            in1=bx[:, None, r0:, 2].to_broadcast(sh3),
                            op=Alu.min,
                        )
                        TT(out=xa, in0=xc, in1=xa, op=Alu.subtract)
                        nc.vector.tensor_scalar_max(xa, xa, 0.0)
                        TT(out=xa, in0=xa, in1=xb, op=Alu.mult)
                        TT(
                            out=xb,
                            in0=d["ar"][:, r0:r0 + RIOU, None].to_broadcast(
                                sh3),
                            in1=d["ar"][:, None, r0:].to_broadcast(sh3),
                            op=Alu.add,
                        )
                        nc.vector.tensor_scalar_mul(xa, xa, 1.45)
                        nc.vector.scalar_tensor_tensor(
                            out=xa, in0=xb, scalar=-0.45, in1=xa,
                            op0=Alu.mult, op1=Alu.add,
                        )
                        nc.vector.tensor_scalar(
                            d["Sm"][:, r0:r0 + RIOU, r0:], xa, 0.0, None,
                            op0=Alu.is_gt,
                        )
                if level < 6:
                    nc.compile()
                    return nc

                # ---- greedy NMS scan (running suppression vector) ----
                # Sm and supv are 0/1, so (1-supv[i])*Sm[i,k] == Sm[i,k] >
                # supv[i]: one fused op per step. supv[:, i] is final after
                # step i-1 (later steps only write k > j >= i), so the final
                # supv vector yields every keep flag in one op afterwards.
                for ti, (off, cnt) in enumerate(TILES):
                    d = D[ti]
                    d["keep"] = sp.tile([cnt, K], u8, tag=f"ke{ti}", name=f"ke{ti}")
                    d["supv"] = sp.tile([cnt, K], u8, tag=f"sv{ti}", name=f"sv{ti}")
                    nc.vector.memset(d["supv"][:, :], 0)
                for i in range(K - 1):
                    for ti, (off, cnt) in enumerate(TILES):
                        d = D[ti]
                        nc.vector.scalar_tensor_tensor(
                            out=d["supv"][:, i + 1:],
                            in0=d["Sm"][:, i, i + 1:],
                            scalar=d["supv"][:, i:i + 1],
                            in1=d["supv"][:, i + 1:],
                            op0=Alu.is_gt, op1=Alu.max,
                        )
                for ti, (off, cnt) in enumerate(TILES):
                    d = D[ti]
                    nc.vector.tensor_scalar(
                        d["keep"][:, :], d["supv"][:, :], 0, None,
                        op0=Alu.is_equal,
                    )
                if level < 7:
                    nc.compile()
                    return nc

                # ---- output scatter ----
                for ti, (off, cnt) in enumerate(TILES):
                    d = D[ti]
                    keepf = sp.tile([cnt, K], f32, tag=f"kf{ti}")
                    nc.vector.tensor_copy(keepf[:, :], d["keep"][:, :])
                    pos = sp.tile([cnt, K], f32, tag=f"pf{ti}")
                    nc.vector.tensor_tensor_scan(
                        out=pos[:, :], data0=keepf[:, :],
                        data1=keepf[:, :], initial=-1.0,
                        op0=Alu.add, op1=Alu.bypass,
                    )
                    posx = sp.tile([cnt, K], f32, tag=f"px{ti}")
                    nc.vector.memset(posx[:, :], float(K))
                    nc.vector.copy_predicated(
                        posx[:, :], d["keep"][:, :], pos[:, :]
                    )
                    posu = sp.tile([cnt, K], u32, tag=f"pu{ti}")
                    nc.vector.tensor_copy(posu[:, :], posx[:, :])
                    # class-0 pairs always go to the trash row
                    cls0 = sp.tile([cnt, K], u32, tag=f"c0{ti}")
                    nc.vector.tensor_scalar(
                        cls0[:, :],
                        d["aux"][:, 1:2].to_broadcast([cnt, K]), K, None,
                        op0=Alu.mult,
                    )
                    TT(out=posu[:, :], in0=posu[:, :], in1=cls0[:, :],
                       op=Alu.max)
                    b3 = sp.tile([cnt, 1], u32, tag=f"b3{ti}")
                    nc.gpsimd.iota(
                        b3[:, :], pattern=[[0, 1]],
                        base=off * 201, channel_multiplier=201,
                    )
                    TT(out=posu[:, :], in0=posu[:, :],
                       in1=b3[:, :].to_broadcast([cnt, K]), op=Alu.add)
                    d["posu"] = posu
                for sg in range(K):
                    for ti, (off, cnt) in enumerate(TILES):
                        d = D[ti]
                        nc.gpsimd.indirect_dma_start(
                            out=outp[:, :],
                            out_offset=IndirectOffsetOnAxis(
                                ap=d["posu"][:, sg:sg + 1], axis=0),
                            in_=d["row"][:, sg, :], in_offset=None,
                        )
    nc.compile()
    return nc


def _prep_core_inputs(loc_b, conf_b, loc2_b, conf2_b, dbox):
    """Pad per-core inputs to NPAD priors; build aux table."""
    pad = NPAD - N
    locp = np.pad(loc_b, ((0, 0), (0, pad), (0, 0)))
    loc2p = np.pad(loc2_b, ((0, 0), (0, pad), (0, 0)))
    cpad = np.zeros((conf_b.shape[0], pad, C), np.float32)
    cpad[:, :, 0] = 40.0
    cpad[:, :, 1:] = -40.0
    confp = np.concatenate([conf_b, cpad], axis=1)
    conf2p = np.concatenate([conf2_b, cpad], axis=1)
    dpad = np.zeros((pad, 4), np.float32)
    dpad[:, 2:] = 1e-3
    dboxp = np.concatenate([dbox, dpad], axis=0)
    aux = np.zeros((PAIRS, 2), np.uint32)
    for p in range(PAIRS):
        aux[p, 0] = (p // C) * W2
        aux[p, 1] = 1 if (p % C) == 0 else 0
    return {
        "loc1": np.ascontiguousarray(locp, np.float32),
        "loc2": np.ascontiguousarray(loc2p, np.float32),
        "conf1": np.ascontiguousarray(confp, np.float32),
        "conf2": np.ascontiguousarray(conf2p, np.float32),
        "dbox": np.ascontiguousarray(dboxp, np.float32),
        "aux": aux,
    }


def kernel(loc_data, conf_data, loc_data2, conf_data2, dbox_list):
    from concourse.bass_utils import run_bass_kernel_spmd

    loc_data = np.asarray(loc_data, np.float32)
    conf_data = np.asarray(conf_data, np.float32)
    loc_data2 = np.asarray(loc_data2, np.float32)
    conf_data2 = np.asarray(conf_data2, np.float32)
    dbox_list = np.asarray(dbox_list, np.float32)

    nc = build_nc()
    in_maps = []
    for k in range(8):
        sl = slice(k * IMG, (k + 1) * IMG)
        in_maps.append(
            _prep_core_inputs(
                loc_data[sl], conf_data[sl], loc_data2[sl], conf_data2[sl],
                dbox_list,
            )
        )
    res = run_bass_kernel_spmd(nc, in_maps, list(range(8))).results
    outs = []
    for k in range(8):
        o = np.asarray(res[k]["out"]).reshape(PAIRS, 201, 5)[:, :K, :]
        outs.append(o.reshape(IMG, C, K, 5))
    return np.concatenate(outs, axis=0)
